# revision 49
# baseline (speedup 1.0000x reference)
"""Trainium2 Bass kernel for ViT-style attention block with RoPE.

Problem: x(64,197,1024), qkv(3072x1024)+b, proj(1024x1024)+b, H=16 heads,
RoPE (interleaved pairs, tiled cos/sin tables) on all tokens but CLS.

Strategy: data-parallel over batch across 8 cores (8 items each, no
collectives). Host pre-transposes all operands so the device only runs
matmuls / softmax / RoPE in "transposed" layouts:

  - qk part:  qkT[f, t] = Wqk^T stationary x xT moving   (features on partitions)
  - v part:   v[t, f]   = xT stationary x Wv moving      (tokens on partitions)
  - scores:   scT[j, i] = kT(lhsT) x qT(rhs); per head one PSUM bank holds
              jt0 (cols 0:S) and jt1 (cols 256:256+S) as two non-accumulating
              full-overwrite matmuls (start=True resets has_written bits for
              the bank but not the data, so disjoint overwrites are safe;
              never pack two ACCUMULATING groups in one bank)
  - softmax:  exp on ScalarE (scale=1/8, no max subtraction; |logits|<~5),
              denominators via ones-columns PREPENDED to v (cols 0:64, so the
              sums land at partition base 0: reciprocal_approx_fast silently
              mis-reads partition-offset inputs on HW), normalization =
              reciprocal + DVE mult (standard DVE ops handle offsets fine)
  - AV:       per head a standard 2-matmul accumulation group in its own bank
  - RoPE:     q' = (q+b)*cos + (P(q+b))*sinS where P is a 128x128 block-swap
              permutation done on the TensorEngine; sign and d-permutation
              folded into host-built tables; elementwise split GpSimd/DVE
  - v bias:   folded into proj bias on host (attn rows sum to 1)
  - proj:     yT = Wproj^T stationary x concatT moving, bias on ScalarE;
              pairs 0..2 project both items in one N=394 chain (each weight
              tile loaded once -> half the LDWEIGHTS), shifted one phase later

Scheduling: a software-pipelined unit queue interleaves QKV/V/proj chain
work between each wave's scores and AV matmuls so the PE never waits on
the ScalarE exp; PSUM is exactly 8 banks: chains 2x[P,512] (bufs=3 incl.
spare), rope 1, scores [P,1024], AV [P,1024]. DRAM layouts make every DMA
contiguous per partition row (fewer descriptors; each dma_start costs
~0.6us of serial DIRECT2D descriptor generation on SyncE, so the startup
stream is ordered x0/fg0 first, then consts, then bulk weights).
"""

import sys
from collections import deque

for _p in ("/opt/trn_rl_repo", "/opt/pypackages"):
    if _p not in sys.path:
        sys.path.append(_p)

import numpy as np
import ml_dtypes

import concourse.bass as bass
import concourse.tile as tile
from concourse import bacc
from concourse import mybir

F32 = mybir.dt.float32
BF16 = mybir.dt.bfloat16
BF16_NP = ml_dtypes.bfloat16

# Problem constants (hardcoded per the contract)
B, N, C = 64, 197, 1024
H, D = 16, 64
E = 1  # CLS tokens
THETA = 10000.0
N_CORES = 8
NI = B // N_CORES  # items per core = 8
NT = NI * N  # tokens per core = 1576
S = N  # 197
W = 2 * S  # pair width = 394
NPAIR = NI // 2  # 4
P = 128


def _host_tables():
    """RoPE cos/sin in device layout + permutations, all position-only."""
    seq = (224 // 16) ** 2  # 196
    exp = np.arange(0, D, 2, dtype=np.float64) / -D
    base = THETA**exp  # (32,)
    t = np.arange(seq, dtype=np.float64)
    f0 = np.outer(t, base)  # (196, 32)
    f = np.concatenate([f0, f0], axis=-1)  # (196, 64) "tiled"
    cos_ref = np.cos(f)
    sin_ref = np.sin(f)

    # permutation: new dd<32 -> orig 2dd (x0), new dd>=32 -> orig 2(dd-32)+1 (x1)
    perm = np.empty(D, dtype=np.int64)
    perm[:32] = np.arange(32) * 2
    perm[32:] = np.arange(32) * 2 + 1

    # per-token columns for an item: col 0 = CLS (cos=1, sin=0), cols 1..196 = rope
    cos_item = np.ones((D, S), dtype=np.float64)
    sin_item = np.zeros((D, S), dtype=np.float64)
    cos_item[:, 1:] = cos_ref[:, perm].T
    sin_item[:, 1:] = sin_ref[:, perm].T
    # fold rotate-half signs into sin: rot[dd<32] = -q[dd+32], rot[dd>=32] = +q[dd-32]
    sinS_item = sin_item.copy()
    sinS_item[:32, :] *= -1.0

    # pair-width, replicated for the 2 heads in a 128-partition tile
    cosT = np.tile(cos_item, (2, 2)).astype(BF16_NP)  # [128, 394]
    sinST = np.tile(sinS_item, (2, 2)).astype(BF16_NP)  # [128, 394]

    # 128x128 swap permutation (block swap +-32 within each 64-head-half),
    # already transposed for use as lhsT: rot = P @ q  ->  lhsT = P.T
    Pm = np.zeros((P, P), dtype=np.float32)
    for p in range(P):
        src = 64 * (p // 64) + ((p % 64) + 32) % 64
        Pm[p, src] = 1.0
    pmatT = Pm.T.astype(BF16_NP)  # [K=128, M=128]

    return perm, cosT, sinST, pmatT


def _pack_weights(qkv_w, qkv_b, proj_w, proj_b, perm):
    """Host-side weight packing into device layouts (all numpy, one-time)."""
    # feature permutation for q/k heads: rows of qkv_w within each head
    qk_perm = np.concatenate(
        [h * D + perm for h in range(2 * H)]  # q heads then k heads
    )
    wqk = qkv_w[:2048][qk_perm]  # (2048, 1024) permuted
    bqk = qkv_b[:2048][qk_perm]  # (2048,)
    wv = qkv_w[2048:]  # (1024, 1024)
    bv = qkv_b[2048:]

    wqk_T = np.ascontiguousarray(wqk.T).astype(BF16_NP)  # [1024 k, 2048 f]
    wv_T = np.ascontiguousarray(wv.T).astype(BF16_NP)  # [1024 k, 1024 f]
    proj_wT = np.ascontiguousarray(proj_w.T).astype(BF16_NP)  # [1024, 1024]

    # device layouts with contiguous per-partition DMA blocks:
    # wqk: [p, fg, o, 256] with k = o*128+p, f = fg*256+c
    wqk_dev = np.ascontiguousarray(
        wqk_T.reshape(8, 128, 8, 256).transpose(1, 2, 0, 3)
    )
    # wv/wpr: [p, o, 1024]
    wv_dev = np.ascontiguousarray(wv_T.reshape(8, 128, 1024).transpose(1, 0, 2))
    wpr_dev = np.ascontiguousarray(proj_wT.reshape(8, 128, 1024).transpose(1, 0, 2))

    # biases in [128, ftile] per-partition layout
    bqk_dev = np.ascontiguousarray(bqk.reshape(16, 128).T).astype(np.float32)
    # v bias folded into proj bias: y = concat@W^T + (W@bv + pb)
    beff = proj_w.astype(np.float64) @ bv.astype(np.float64) + proj_b
    beff_dev = np.ascontiguousarray(beff.reshape(8, 128).T).astype(np.float32)
    return wqk_dev, wv_dev, wpr_dev, bqk_dev, beff_dev


def build_nc(n_items=NI, debug_taps=False):
    """Build the per-core Bass graph. SPMD: same graph on all cores."""
    assert n_items == NI
    nc = bacc.Bacc(None, target_bir_lowering=False, debug=False)

    # all DRAM params are 2-D [P, cols]; >2-D shapes are realized as device-
    # side rearranges (the HW ingestion path lays out 2-D exactly like the
    # host's C-order arrays)
    xT2 = nc.declare_dram_parameter("xT", [P, NPAIR * 8 * W], BF16, isOutput=False)
    wqk2 = nc.declare_dram_parameter("wqk", [P, 8 * 8 * 256], BF16, isOutput=False)
    wv2 = nc.declare_dram_parameter("wv", [P, 8 * C], BF16, isOutput=False)
    wpr2 = nc.declare_dram_parameter("wpr", [P, 8 * C], BF16, isOutput=False)
    pmat = nc.declare_dram_parameter("pmat", [P, P], BF16, isOutput=False)
    bqk = nc.declare_dram_parameter("bqk", [P, 16], F32, isOutput=False)
    beff = nc.declare_dram_parameter("beff", [P, 8], F32, isOutput=False)
    cosT = nc.declare_dram_parameter("cosT", [P, W], BF16, isOutput=False)
    sinST = nc.declare_dram_parameter("sinST", [P, W], BF16, isOutput=False)
    out2 = nc.declare_dram_parameter(
        "out", [P, NPAIR * 2 * 8 * S], F32, isOutput=True
    )
    if debug_taps:
        dbg_roped = nc.declare_dram_parameter("dbg_roped", [P, 16 * W], BF16, True)
        dbg_v = nc.declare_dram_parameter("dbg_v", [P, 16 * 128], BF16, True)
        dbg_concat = nc.declare_dram_parameter("dbg_concat", [P, 8 * W], BF16, True)
        dbg_e = nc.declare_dram_parameter("dbg_e", [P, S], BF16, True)
    xT = xT2.rearrange("p (pr k w) -> p pr k w", pr=NPAIR, k=8)
    wqk = wqk2.rearrange("p (fg o c) -> p fg o c", fg=8, o=8)
    wv = wv2.rearrange("p (o f) -> p o f", o=8)
    wpr = wpr2.rearrange("p (o f) -> p o f", o=8)
    out = out2.rearrange("p (pr o it s) -> p pr o it s", pr=NPAIR, o=8, it=2)

    Exp = mybir.ActivationFunctionType.Exp
    Ident = mybir.ActivationFunctionType.Identity

    with tile.TileContext(nc) as tc:
        with (
            tc.tile_pool(name="const", bufs=1) as const,
            tc.tile_pool(name="xp", bufs=2) as xp,
            tc.tile_pool(name="roped", bufs=2) as rp,
            tc.tile_pool(name="vp", bufs=2) as vp,
            tc.tile_pool(name="work", bufs=2) as wk,
            tc.tile_pool(name="ep", bufs=2) as ep,
            tc.tile_pool(name="cc", bufs=2) as cc,
            tc.tile_pool(name="yp", bufs=2) as yp,
            tc.tile_pool(name="psC", bufs=3, space="PSUM") as psC,
            tc.tile_pool(name="psR", bufs=1, space="PSUM") as psR,
            tc.tile_pool(name="psS", bufs=1, space="PSUM") as psS,
            tc.tile_pool(name="psV", bufs=1, space="PSUM") as psV,
        ):
            # ---- early DMAs: the first chain's inputs (x0, fg0) lead, then
            # the consts (needed ~2us later by the first IDENT/rope), then
            # the rest of the wqk stream and v/proj weights ----
            x_first = xp.tile([P, 8, W], BF16, tag="x")
            wqk_sb = const.tile([P, 8, 8, 256], BF16)
            nc.sync.dma_start(x_first, xT[:, 0])
            nc.sync.dma_start(wqk_sb[:, 0], wqk[:, 0])

            bqk_sb = const.tile([P, 16], F32)
            nc.sync.dma_start(bqk_sb, bqk[:, :])
            beff_sb = const.tile([P, 8], F32)
            nc.sync.dma_start(beff_sb, beff[:, :])
            cos_sb = const.tile([P, W], BF16)
            nc.sync.dma_start(cos_sb, cosT[:, :])
            sin_sb = const.tile([P, W], BF16)
            nc.sync.dma_start(sin_sb, sinST[:, :])
            pmat_sb = const.tile([P, P], BF16)
            nc.sync.dma_start(pmat_sb, pmat[:, :])

            # warm the activation table so the 1.3us table load is off the
            # critical path of the first bias-add
            dummy = const.tile([P, 8], F32)
            nc.gpsimd.memset(dummy, 0.0)
            dummy2 = const.tile([P, 8], F32)
            nc.scalar.activation(dummy2, dummy, Exp, scale=0.125)

            for fg in range(1, 8):
                nc.sync.dma_start(wqk_sb[:, fg], wqk[:, fg])
            wv_sb = const.tile([P, 8, C], BF16)
            nc.sync.dma_start(wv_sb, wv[:, :, :])
            wpr_sb = const.tile([P, 8, C], BF16)
            nc.sync.dma_start(wpr_sb, wpr[:, :, :])

            # ---- emission helpers ----
            def emit_qk_ft(x_sb, roped, ft):
                ps = psC.tile([P, 512], F32, tag="chain")
                fg, half = ft // 2, ft % 2
                for kt in range(8):
                    nc.tensor.matmul(
                        ps[:, 0:W],
                        wqk_sb[:, fg, kt, half * 128 : (half + 1) * 128],
                        x_sb[:, kt, :],
                        start=(kt == 0),
                        stop=(kt == 7),
                    )
                tmp = wk.tile([P, W], BF16, tag="tmp")
                nc.scalar.activation(
                    tmp, ps[:, 0:W], Ident, bias=bqk_sb[:, ft : ft + 1]
                )
                psr = psR.tile([P, 512], F32, tag="rope")
                nc.tensor.matmul(psr[:, 0:W], pmat_sb, tmp, start=True, stop=True)
                acc = wk.tile([P, W], BF16, tag="acc")
                nc.gpsimd.tensor_mul(acc, tmp, cos_sb)
                rot2 = wk.tile([P, W], BF16, tag="rot2")
                nc.vector.tensor_mul(rot2, psr[:, 0:W], sin_sb)
                nc.gpsimd.tensor_add(roped[:, ft, :], acc, rot2)

            def make_vt(it2, tt):
                pcount = 128 if tt == 0 else 69
                vt = vp.tile([P, 16, 128], BF16, tag=f"v{it2}{tt}")
                return vt

            def emit_v_nk(x_sb, vt, it2, tt, nk, kts=range(8), ps=None):
                pcount = 128 if tt == 0 else 69
                kts = list(kts)
                if ps is None:
                    ps = psC.tile([P, 512], F32, tag="chain")
                for kt in kts:
                    nc.tensor.matmul(
                        ps[:pcount, :],
                        x_sb[:, kt, it2 * S + tt * P : it2 * S + tt * P + pcount],
                        wv_sb[:, kt, nk * 512 : (nk + 1) * 512],
                        start=(kt == 0),
                        stop=(kt == 7),
                    )
                if kts[-1] != 7:
                    return ps
                # ones in cols 0:64, features in 64:128 so the AV output has
                # the softmax sums at partition base 0 (reciprocal_approx_fast
                # mis-reads partition-offset inputs on HW)
                nc.vector.tensor_copy(
                    vt[:pcount, nk * 8 : (nk + 1) * 8, 64:128],
                    ps[:pcount, :].rearrange("p (h d) -> p h d", d=64),
                )
                if nk == 1:
                    nc.gpsimd.memset(vt[:pcount, :, 0:64], 1.0)

            def emit_proj_fused(concat, y, ft, kts=range(8), ps=None):
                """proj for BOTH items of a pair at once: N=W columns, so
                each weight tile is loaded once for 394 output columns
                (halves LDWEIGHTS traffic vs per-item chains)."""
                kts = list(kts)
                if ps is None:
                    ps = psC.tile([P, 512], F32, tag="chain")
                for kt in kts:
                    nc.tensor.matmul(
                        ps[:, 0:W],
                        wpr_sb[:, kt, ft * 128 : (ft + 1) * 128],
                        concat[:, kt, :],
                        start=(kt == 0),
                        stop=(kt == 7),
                    )
                if kts[-1] == 7:
                    nc.scalar.activation(
                        y[:, ft, :], ps[:, 0:W], Ident, bias=beff_sb[:, ft : ft + 1]
                    )
                return ps

            def emit_proj(concat, y, it2, ft, kts=range(8), ps=None):
                ts = it2 * S
                kts = list(kts)
                if ps is None:
                    ps = psC.tile([P, 512], F32, tag="chain")
                for kt in kts:
                    nc.tensor.matmul(
                        ps[:, 0:S],
                        wpr_sb[:, kt, ft * 128 : (ft + 1) * 128],
                        concat[:, kt, ts : ts + S],
                        start=(kt == 0),
                        stop=(kt == 7),
                    )
                if kts[-1] == 7:
                    nc.scalar.activation(
                        y[:, ft, it2, :], ps[:, 0:S], Ident,
                        bias=beff_sb[:, ft : ft + 1],
                    )
                return ps

            def emit_wave_sc(roped, it2, hp):
                """Scores for head pair hp. Per head one PSUM bank holding
                jt0 at cols 0:S and jt1 at cols 256:256+S. Both writes are
                full overwrites of disjoint ranges (start=True stop=True),
                never accumulation, so sharing the bank is safe on HW (start
                clears has_written bits, not data)."""
                ts = it2 * S
                sc = psS.tile([P, 1024], F32, tag="sc")
                kqs = []
                for bk, h in ((0, 2 * hp), (1, 2 * hp + 1)):
                    hb = 64 * (h % 2)
                    kT = roped[hb : hb + 64, 8 + h // 2, ts : ts + S]
                    qT = roped[hb : hb + 64, h // 2, ts : ts + S]
                    kqs.append((bk, kT, qT))
                # jt0 of both heads first so e0 is ready one matmul earlier
                for bk, kT, qT in kqs:
                    nc.tensor.matmul(
                        sc[:, bk * 512 : bk * 512 + S], kT[:, 0:P], qT,
                        start=True, stop=True, skip_group_check=True,
                    )
                for bk, kT, qT in kqs:
                    nc.tensor.matmul(
                        sc[0:69, bk * 512 + 256 : bk * 512 + 256 + S],
                        kT[:, P:S], qT,
                        start=True, stop=True, skip_group_check=True,
                    )
                sc4 = sc.rearrange("p (b c) -> p b c", b=2)
                e0 = ep.tile([P, 2, S], BF16, tag="e0")
                e1 = ep.tile([P, 2, S], BF16, tag="e1")
                nc.scalar.activation(e0, sc4[:, :, 0:S], Exp, scale=0.125)
                nc.scalar.activation(
                    e1[0:69], sc4[0:69, :, 256 : 256 + S], Exp, scale=0.125
                )
                return e0, e1

            def emit_wave_av(v65, concat, e0, e1, it2, hp):
                """AV + normalization for a head pair; per head one standard
                2-matmul accumulation group in its own bank."""
                ts = it2 * S
                hA, hB = 2 * hp, 2 * hp + 1
                av = psV.tile([P, 1024], F32, tag="av")
                nc.tensor.matmul(
                    av[:, 0:S], v65[it2][0][:, hA, :], e0[:, 0],
                    start=True, stop=False,
                )
                nc.tensor.matmul(
                    av[:, 512 : 512 + S], v65[it2][0][:, hB, :], e0[:, 1],
                    start=True, stop=False,
                )
                nc.tensor.matmul(
                    av[:, 0:S], v65[it2][1][0:69, hA, :], e1[0:69, 0],
                    start=False, stop=True,
                )
                nc.tensor.matmul(
                    av[:, 512 : 512 + S], v65[it2][1][0:69, hB, :], e1[0:69, 1],
                    start=False, stop=True,
                )
                av4 = av.rearrange("p (b c) -> p b c", b=2)
                rb = wk.tile([64, 2, S], F32, tag="rb")
                nc.vector.reciprocal_approx_fast(rb, av4[0:64, :, 0:S])
                nc.vector.tensor_mul(
                    concat[0:64, hp, ts : ts + S], av4[64:128, 0, 0:S], rb[:, 0]
                )
                nc.vector.tensor_mul(
                    concat[64:128, hp, ts : ts + S], av4[64:128, 1, 0:S], rb[:, 1]
                )

            # ---- prologue: pair-0 QKV + V(it0), bare (DMA-paced anyway) ----
            roped_cur = rp.tile([P, 16, W], BF16)
            for ft in range(16):
                emit_qk_ft(x_first, roped_cur, ft)
            v65_cur = [[None, None], [None, None]]
            for tt in range(2):
                v65_cur[0][tt] = make_vt(0, tt)
                for nk in range(2):
                    emit_v_nk(x_first, v65_cur[0][tt], 0, tt, nk)

            # ---- software-pipelined pair phases with a filler-unit queue ----
            queue = deque()
            x_cur = x_first

            def enq_split(fn):
                """Enqueue a chain as two 4-matmul halves sharing one psum
                tile (finer filler granularity for slot-starved phases)."""
                sh = {}

                def a():
                    sh["ps"] = fn(kts=range(0, 4))

                def b():
                    fn(kts=range(4, 8), ps=sh["ps"])

                queue.append(a)
                queue.append(b)

            def enq_v_units(x_sb, v65dst, split):
                for it2 in range(2):
                    for tt in range(2):
                        v65dst[it2][tt] = make_vt(it2, tt)
                        for nk in range(2):
                            fn = (
                                lambda x=x_sb, vt=v65dst[it2][tt], i=it2,
                                t=tt, n=nk, **kw: emit_v_nk(x, vt, i, t, n, **kw)
                            )
                            if split:
                                enq_split(fn)
                            else:
                                queue.append(fn)

            concat_prev = None
            yF_prev = None

            for pr in range(NPAIR):
                nxt = pr + 1 if pr + 1 < NPAIR else None
                concat = cc.tile([P, 8, W], BF16)

                if pr == 0:
                    # v65 for pair-0 it1, consumed by the first it0 pops
                    for tt in range(2):
                        v65_cur[1][tt] = make_vt(1, tt)
                        for nk in range(2):
                            queue.append(
                                (lambda x=x_cur, vt=v65_cur[1][tt], t=tt, n=nk:
                                 emit_v_nk(x, vt, 1, t, n))
                            )

                if nxt is not None:
                    x_nxt = xp.tile([P, 8, W], BF16, tag="x")
                    nc.sync.dma_start(x_nxt, xT[:, nxt])
                    roped_nxt = rp.tile([P, 16, W], BF16)
                    for ft in range(16):
                        queue.append(
                            (lambda x=x_nxt, r=roped_nxt, f=ft:
                             emit_qk_ft(x, r, f))
                        )
                else:
                    # final phase it0 filler: fused proj of the previous pair,
                    # split into half-chains to cover all 16 slots
                    yF = yp.tile([P, 8, W], F32, tag="yF", bufs=1)
                    for ft in range(8):
                        enq_split(
                            lambda c=concat_prev, y=yF, f=ft, **kw:
                            emit_proj_fused(c, y, f, **kw)
                        )
                    yF_prev = yF

                # it0 waves (16 pops, filler units between scores and AV)
                for hp in range(8):
                    e0, e1 = emit_wave_sc(roped_cur, 0, hp)
                    if queue:
                        queue.popleft()()
                    emit_wave_av(v65_cur, concat, e0, e1, 0, hp)
                    if queue:
                        queue.popleft()()

                if nxt is not None:
                    v65_nxt = [[None, None], [None, None]]
                    enq_v_units(x_nxt, v65_nxt, split=(pr == 0))
                    if pr >= 1:
                        # fused proj of pair pr-1 (whole chains)
                        yF = yp.tile([P, 8, W], F32, tag="yF", bufs=1)
                        for ft in range(8):
                            queue.append(
                                (lambda c=concat_prev, y=yF, f=ft:
                                 emit_proj_fused(c, y, f))
                            )
                        yF_prev = yF
                else:
                    v65_nxt = None
                    # deferred pair-2 output is fully emitted by now
                    nc.sync.dma_start(
                        out[:, NPAIR - 2],
                        yF_prev.rearrange("p f (it s) -> p f it s", it=2),
                    )
                    # final phase it1 filler: this pair's it0 proj, split
                    y3 = yp.tile([P, 8, 2, S], F32, tag="y3", bufs=1)
                    for ft in range(8):
                        enq_split(
                            lambda c=concat, y=y3, f=ft, **kw:
                            emit_proj(c, y, 0, f, **kw)
                        )

                # it1 waves (16 pops)
                for hp in range(8):
                    e0, e1 = emit_wave_sc(roped_cur, 1, hp)
                    if queue:
                        queue.popleft()()
                    emit_wave_av(v65_cur, concat, e0, e1, 1, hp)
                    if queue:
                        queue.popleft()()

                # drain leftovers (pure chain work, gap-free)
                while queue:
                    queue.popleft()()

                if debug_taps and pr == 0:
                    nc.sync.dma_start(
                        dbg_roped.rearrange("p (f w) -> p f w", f=16), roped_cur
                    )
                    nc.sync.dma_start(
                        dbg_v.rearrange("p (h d) -> p h d", h=16), v65_cur[0][0]
                    )
                    nc.sync.dma_start(
                        dbg_concat.rearrange("p (j w) -> p j w", j=8), concat
                    )
                    nc.sync.dma_start(dbg_e[:, :], e0[:, 0])

                if nxt is not None:
                    if pr >= 1:
                        # pair pr-1's fused output is complete after the drain
                        nc.sync.dma_start(
                            out[:, pr - 1],
                            yF_prev.rearrange("p f (it s) -> p f it s", it=2),
                        )
                    concat_prev = concat
                    x_cur, roped_cur, v65_cur = x_nxt, roped_nxt, v65_nxt
                else:
                    # tail: this pair's it0 output, then bare it1 proj chains
                    # with the output streamed out in 2-ft chunks so the last
                    # DMA only moves ~200KB after the final IDENT
                    nc.sync.dma_start(out[:, pr, :, 0, :], y3[:, :, 0])
                    for ft in range(8):
                        emit_proj(concat, y3, 1, ft)
                        if ft % 2 == 1:
                            nc.sync.dma_start(
                                out[:, pr, ft - 1 : ft + 1, 1, :],
                                y3[:, ft - 1 : ft + 1, 1],
                            )

    nc.compile()
    return nc


def host_pack_inputs(x, qkv_w, qkv_b, proj_w, proj_b, n_items=NI):
    """Build per-core in_maps (host-side layout only, no math on x)."""
    perm, cosT, sinST, pmatT = _host_tables()
    wqk_dev, wv_dev, wpr_dev, bqk_dev, beff_dev = _pack_weights(
        qkv_w, qkv_b, proj_w, proj_b, perm
    )
    shared = {
        "wqk": np.ascontiguousarray(wqk_dev.reshape(P, -1)),
        "wv": np.ascontiguousarray(wv_dev.reshape(P, -1)),
        "wpr": np.ascontiguousarray(wpr_dev.reshape(P, -1)),
        "pmat": np.ascontiguousarray(pmatT),
        "bqk": bqk_dev,
        "beff": beff_dev,
        "cosT": np.ascontiguousarray(cosT),
        "sinST": np.ascontiguousarray(sinST),
    }
    n_cores = x.shape[0] // n_items
    in_maps = []
    for c in range(n_cores):
        xs = x[c * n_items : (c + 1) * n_items]  # [ni, 197, 1024]
        # device layout [p, pair, kt, w]: feature kt*128+p, token pair*394+w
        xt = xs.reshape(NPAIR, W, C).astype(BF16_NP)
        xt = xt.transpose(2, 0, 1).reshape(8, 128, NPAIR, W).transpose(1, 2, 0, 3)
        in_maps.append(
            {"xT": np.ascontiguousarray(xt.reshape(P, -1)), **shared}
        )
    return in_maps


def unpack_output(results, n_items=NI):
    """results: per-core {'out': [128, NPAIR, 2, 8, S]} -> full (B, N, C)."""
    outs = []
    for r in results:
        yT = r["out"].reshape(P, NPAIR, 8, 2, S)
        # feature o*128+p, token pr*394+it*197+s; device layout [p,pr,o,it,s]
        y = yT.transpose(1, 3, 4, 2, 0).reshape(n_items * S, C)
        outs.append(y.reshape(n_items, S, C))
    return np.concatenate(outs, axis=0)


_CACHED = {}


def kernel(x, qkv_w, qkv_b, proj_w, proj_b):
    from concourse.bass_utils import run_bass_kernel_spmd

    x = np.asarray(x, dtype=np.float32)
    qkv_w = np.asarray(qkv_w, dtype=np.float32)
    qkv_b = np.asarray(qkv_b, dtype=np.float32)
    proj_w = np.asarray(proj_w, dtype=np.float32)
    proj_b = np.asarray(proj_b, dtype=np.float32)

    if "nc" not in _CACHED:
        _CACHED["nc"] = build_nc(NI)
    nc = _CACHED["nc"]
    in_maps = host_pack_inputs(x, qkv_w, qkv_b, proj_w, proj_b, NI)
    res = run_bass_kernel_spmd(nc, in_maps, core_ids=list(range(N_CORES)))
    return unpack_output(res.results, NI).astype(np.float32)


if __name__ == "__main__":
    pass


# revision 50
# speedup vs baseline: 1.0044x; 1.0044x over previous
"""Trainium2 Bass kernel for ViT-style attention block with RoPE.

Problem: x(64,197,1024), qkv(3072x1024)+b, proj(1024x1024)+b, H=16 heads,
RoPE (interleaved pairs, tiled cos/sin tables) on all tokens but CLS.

Strategy: data-parallel over batch across 8 cores (8 items each, no
collectives). Host pre-transposes all operands so the device only runs
matmuls / softmax / RoPE in "transposed" layouts:

  - qk part:  qkT[f, t] = Wqk^T stationary x xT moving   (features on partitions)
  - v part:   v[t, f]   = xT stationary x Wv moving      (tokens on partitions)
  - scores:   scT[j, i] = kT(lhsT) x qT(rhs); per head one PSUM bank holds
              jt0 (cols 0:S) and jt1 (cols 256:256+S) as two non-accumulating
              full-overwrite matmuls (start=True resets has_written bits for
              the bank but not the data, so disjoint overwrites are safe;
              never pack two ACCUMULATING groups in one bank)
  - softmax:  exp on ScalarE (scale=1/8, no max subtraction; |logits|<~5),
              denominators via ones-columns PREPENDED to v (cols 0:64, so the
              sums land at partition base 0: reciprocal_approx_fast silently
              mis-reads partition-offset inputs on HW), normalization =
              reciprocal + DVE mult (standard DVE ops handle offsets fine)
  - AV:       per head a standard 2-matmul accumulation group in its own bank
  - RoPE:     q' = (q+b)*cos + (P(q+b))*sinS where P is a 128x128 block-swap
              permutation done on the TensorEngine; sign and d-permutation
              folded into host-built tables; elementwise split GpSimd/DVE
  - v bias:   folded into proj bias on host (attn rows sum to 1)
  - proj:     yT = Wproj^T stationary x concatT moving, bias on ScalarE;
              pairs 0..2 project both items in one N=394 chain (each weight
              tile loaded once -> half the LDWEIGHTS), shifted one phase later

Scheduling: a software-pipelined unit queue interleaves QKV/V/proj chain
work between each wave's scores and AV matmuls so the PE never waits on
the ScalarE exp; PSUM is exactly 8 banks: chains 2x[P,512] (bufs=3 incl.
spare), rope 1, scores [P,1024], AV [P,1024]. DRAM layouts make every DMA
contiguous per partition row (fewer descriptors; each dma_start costs
~0.6us of serial DIRECT2D descriptor generation on SyncE, so the startup
stream is ordered x0/fg0 first, then consts, then bulk weights).
"""

import sys
from collections import deque

for _p in ("/opt/trn_rl_repo", "/opt/pypackages"):
    if _p not in sys.path:
        sys.path.append(_p)

import numpy as np
import ml_dtypes

import concourse.bass as bass
import concourse.tile as tile
from concourse import bacc
from concourse import mybir

F32 = mybir.dt.float32
BF16 = mybir.dt.bfloat16
BF16_NP = ml_dtypes.bfloat16

# Problem constants (hardcoded per the contract)
B, N, C = 64, 197, 1024
H, D = 16, 64
E = 1  # CLS tokens
THETA = 10000.0
N_CORES = 8
NI = B // N_CORES  # items per core = 8
NT = NI * N  # tokens per core = 1576
S = N  # 197
W = 2 * S  # pair width = 394
NPAIR = NI // 2  # 4
P = 128


def _host_tables():
    """RoPE cos/sin in device layout + permutations, all position-only."""
    seq = (224 // 16) ** 2  # 196
    exp = np.arange(0, D, 2, dtype=np.float64) / -D
    base = THETA**exp  # (32,)
    t = np.arange(seq, dtype=np.float64)
    f0 = np.outer(t, base)  # (196, 32)
    f = np.concatenate([f0, f0], axis=-1)  # (196, 64) "tiled"
    cos_ref = np.cos(f)
    sin_ref = np.sin(f)

    # permutation: new dd<32 -> orig 2dd (x0), new dd>=32 -> orig 2(dd-32)+1 (x1)
    perm = np.empty(D, dtype=np.int64)
    perm[:32] = np.arange(32) * 2
    perm[32:] = np.arange(32) * 2 + 1

    # per-token columns for an item: col 0 = CLS (cos=1, sin=0), cols 1..196 = rope
    cos_item = np.ones((D, S), dtype=np.float64)
    sin_item = np.zeros((D, S), dtype=np.float64)
    cos_item[:, 1:] = cos_ref[:, perm].T
    sin_item[:, 1:] = sin_ref[:, perm].T
    # fold rotate-half signs into sin: rot[dd<32] = -q[dd+32], rot[dd>=32] = +q[dd-32]
    sinS_item = sin_item.copy()
    sinS_item[:32, :] *= -1.0

    # pair-width, replicated for the 2 heads in a 128-partition tile
    cosT = np.tile(cos_item, (2, 2)).astype(BF16_NP)  # [128, 394]
    sinST = np.tile(sinS_item, (2, 2)).astype(BF16_NP)  # [128, 394]

    # 128x128 swap permutation (block swap +-32 within each 64-head-half),
    # already transposed for use as lhsT: rot = P @ q  ->  lhsT = P.T
    Pm = np.zeros((P, P), dtype=np.float32)
    for p in range(P):
        src = 64 * (p // 64) + ((p % 64) + 32) % 64
        Pm[p, src] = 1.0
    pmatT = Pm.T.astype(BF16_NP)  # [K=128, M=128]

    return perm, cosT, sinST, pmatT


def _pack_weights(qkv_w, qkv_b, proj_w, proj_b, perm):
    """Host-side weight packing into device layouts (all numpy, one-time)."""
    # feature permutation for q/k heads: rows of qkv_w within each head
    qk_perm = np.concatenate(
        [h * D + perm for h in range(2 * H)]  # q heads then k heads
    )
    wqk = qkv_w[:2048][qk_perm]  # (2048, 1024) permuted
    bqk = qkv_b[:2048][qk_perm]  # (2048,)
    wv = qkv_w[2048:]  # (1024, 1024)
    bv = qkv_b[2048:]

    wqk_T = np.ascontiguousarray(wqk.T).astype(BF16_NP)  # [1024 k, 2048 f]
    wv_T = np.ascontiguousarray(wv.T).astype(BF16_NP)  # [1024 k, 1024 f]
    proj_wT = np.ascontiguousarray(proj_w.T).astype(BF16_NP)  # [1024, 1024]

    # device layouts with contiguous per-partition DMA blocks:
    # wqk: [p, fg, o, 256] with k = o*128+p, f = fg*256+c
    wqk_dev = np.ascontiguousarray(
        wqk_T.reshape(8, 128, 8, 256).transpose(1, 2, 0, 3)
    )
    # wv/wpr: [p, o, 1024]
    wv_dev = np.ascontiguousarray(wv_T.reshape(8, 128, 1024).transpose(1, 0, 2))
    wpr_dev = np.ascontiguousarray(proj_wT.reshape(8, 128, 1024).transpose(1, 0, 2))

    # biases in [128, ftile] per-partition layout
    bqk_dev = np.ascontiguousarray(bqk.reshape(16, 128).T).astype(np.float32)
    # v bias folded into proj bias: y = concat@W^T + (W@bv + pb)
    beff = proj_w.astype(np.float64) @ bv.astype(np.float64) + proj_b
    beff_dev = np.ascontiguousarray(beff.reshape(8, 128).T).astype(np.float32)
    return wqk_dev, wv_dev, wpr_dev, bqk_dev, beff_dev


def build_nc(n_items=NI, debug_taps=False):
    """Build the per-core Bass graph. SPMD: same graph on all cores."""
    assert n_items == NI
    nc = bacc.Bacc(None, target_bir_lowering=False, debug=False)

    # all DRAM params are 2-D [P, cols]; >2-D shapes are realized as device-
    # side rearranges (the HW ingestion path lays out 2-D exactly like the
    # host's C-order arrays)
    xT2 = nc.declare_dram_parameter("xT", [P, NPAIR * 8 * W], BF16, isOutput=False)
    wqk2 = nc.declare_dram_parameter("wqk", [P, 8 * 8 * 256], BF16, isOutput=False)
    wv2 = nc.declare_dram_parameter("wv", [P, 8 * C], BF16, isOutput=False)
    wpr2 = nc.declare_dram_parameter("wpr", [P, 8 * C], BF16, isOutput=False)
    pmat = nc.declare_dram_parameter("pmat", [P, P], BF16, isOutput=False)
    bqk = nc.declare_dram_parameter("bqk", [P, 16], F32, isOutput=False)
    beff = nc.declare_dram_parameter("beff", [P, 8], F32, isOutput=False)
    cosT = nc.declare_dram_parameter("cosT", [P, W], BF16, isOutput=False)
    sinST = nc.declare_dram_parameter("sinST", [P, W], BF16, isOutput=False)
    out2 = nc.declare_dram_parameter(
        "out", [P, NPAIR * 2 * 8 * S], F32, isOutput=True
    )
    if debug_taps:
        dbg_roped = nc.declare_dram_parameter("dbg_roped", [P, 16 * W], BF16, True)
        dbg_v = nc.declare_dram_parameter("dbg_v", [P, 16 * 128], BF16, True)
        dbg_concat = nc.declare_dram_parameter("dbg_concat", [P, 8 * W], BF16, True)
        dbg_e = nc.declare_dram_parameter("dbg_e", [P, S], BF16, True)
    xT = xT2.rearrange("p (pr k w) -> p pr k w", pr=NPAIR, k=8)
    wqk = wqk2.rearrange("p (fg o c) -> p fg o c", fg=8, o=8)
    wv = wv2.rearrange("p (o f) -> p o f", o=8)
    wpr = wpr2.rearrange("p (o f) -> p o f", o=8)
    out = out2.rearrange("p (pr o it s) -> p pr o it s", pr=NPAIR, o=8, it=2)

    Exp = mybir.ActivationFunctionType.Exp
    Ident = mybir.ActivationFunctionType.Identity

    with tile.TileContext(nc) as tc:
        with (
            tc.tile_pool(name="const", bufs=1) as const,
            tc.tile_pool(name="xp", bufs=2) as xp,
            tc.tile_pool(name="roped", bufs=2) as rp,
            tc.tile_pool(name="vp", bufs=2) as vp,
            tc.tile_pool(name="workp", bufs=2) as wk,
            tc.tile_pool(name="ep", bufs=2) as ep,
            tc.tile_pool(name="cc", bufs=2) as cc,
            tc.tile_pool(name="yp", bufs=2) as yp,
            tc.tile_pool(name="psC", bufs=3, space="PSUM") as psC,
            tc.tile_pool(name="psR", bufs=1, space="PSUM") as psR,
            tc.tile_pool(name="psS", bufs=1, space="PSUM") as psS,
            tc.tile_pool(name="psV", bufs=1, space="PSUM") as psV,
        ):
            # ---- early DMAs: the first chain's inputs (x0, fg0) lead, then
            # the consts (needed ~2us later by the first IDENT/rope), then
            # the rest of the wqk stream and v/proj weights ----
            x_first = xp.tile([P, 8, W], BF16, tag="x")
            wqk_sb = const.tile([P, 8, 8, 256], BF16)
            nc.sync.dma_start(x_first, xT[:, 0])
            nc.sync.dma_start(wqk_sb[:, 0], wqk[:, 0])

            bqk_sb = const.tile([P, 16], F32)
            nc.sync.dma_start(bqk_sb, bqk[:, :])
            beff_sb = const.tile([P, 8], F32)
            nc.sync.dma_start(beff_sb, beff[:, :])
            cos_sb = const.tile([P, W], BF16)
            nc.sync.dma_start(cos_sb, cosT[:, :])
            sin_sb = const.tile([P, W], BF16)
            nc.sync.dma_start(sin_sb, sinST[:, :])
            pmat_sb = const.tile([P, P], BF16)
            nc.sync.dma_start(pmat_sb, pmat[:, :])

            # warm the activation table so the 1.3us table load is off the
            # critical path of the first bias-add
            dummy = const.tile([P, 8], F32)
            nc.gpsimd.memset(dummy, 0.0)
            dummy2 = const.tile([P, 8], F32)
            nc.scalar.activation(dummy2, dummy, Exp, scale=0.125)

            for fg in range(1, 8):
                nc.sync.dma_start(wqk_sb[:, fg], wqk[:, fg])
            wv_sb = const.tile([P, 8, C], BF16)
            nc.sync.dma_start(wv_sb, wv[:, :, :])
            wpr_sb = const.tile([P, 8, C], BF16)
            nc.sync.dma_start(wpr_sb, wpr[:, :, :])

            # ---- emission helpers ----
            def emit_qk_ft(x_sb, roped, ft):
                ps = psC.tile([P, 512], F32, tag="chain")
                fg, half = ft // 2, ft % 2
                for kt in range(8):
                    nc.tensor.matmul(
                        ps[:, 0:W],
                        wqk_sb[:, fg, kt, half * 128 : (half + 1) * 128],
                        x_sb[:, kt, :],
                        start=(kt == 0),
                        stop=(kt == 7),
                    )
                tmp = wk.tile([P, W], BF16, tag="tmp")
                nc.scalar.activation(
                    tmp, ps[:, 0:W], Ident, bias=bqk_sb[:, ft : ft + 1]
                )
                psr = psR.tile([P, 512], F32, tag="rope")
                nc.tensor.matmul(psr[:, 0:W], pmat_sb, tmp, start=True, stop=True)
                acc = wk.tile([P, W], BF16, tag="acc")
                nc.gpsimd.tensor_mul(acc, tmp, cos_sb)
                rot2 = wk.tile([P, W], BF16, tag="rot2")
                nc.vector.tensor_mul(rot2, psr[:, 0:W], sin_sb)
                nc.gpsimd.tensor_add(roped[:, ft, :], acc, rot2)

            def make_vt(it2, tt):
                pcount = 128 if tt == 0 else 69
                vt = vp.tile([P, 16, 128], BF16, tag=f"v{it2}{tt}")
                return vt

            def emit_v_nk(x_sb, vt, it2, tt, nk, kts=range(8), ps=None):
                pcount = 128 if tt == 0 else 69
                kts = list(kts)
                if ps is None:
                    ps = psC.tile([P, 512], F32, tag="chain")
                for kt in kts:
                    nc.tensor.matmul(
                        ps[:pcount, :],
                        x_sb[:, kt, it2 * S + tt * P : it2 * S + tt * P + pcount],
                        wv_sb[:, kt, nk * 512 : (nk + 1) * 512],
                        start=(kt == 0),
                        stop=(kt == 7),
                    )
                if kts[-1] != 7:
                    return ps
                # ones in cols 0:64, features in 64:128 so the AV output has
                # the softmax sums at partition base 0 (reciprocal_approx_fast
                # mis-reads partition-offset inputs on HW)
                nc.vector.tensor_copy(
                    vt[:pcount, nk * 8 : (nk + 1) * 8, 64:128],
                    ps[:pcount, :].rearrange("p (h d) -> p h d", d=64),
                )
                if nk == 1:
                    nc.gpsimd.memset(vt[:pcount, :, 0:64], 1.0)

            def emit_proj_fused(concat, y, ft, kts=range(8), ps=None):
                """proj for BOTH items of a pair at once: N=W columns, so
                each weight tile is loaded once for 394 output columns
                (halves LDWEIGHTS traffic vs per-item chains)."""
                kts = list(kts)
                if ps is None:
                    ps = psC.tile([P, 512], F32, tag="chain")
                for kt in kts:
                    nc.tensor.matmul(
                        ps[:, 0:W],
                        wpr_sb[:, kt, ft * 128 : (ft + 1) * 128],
                        concat[:, kt, :],
                        start=(kt == 0),
                        stop=(kt == 7),
                    )
                if kts[-1] == 7:
                    nc.scalar.activation(
                        y[:, ft, :], ps[:, 0:W], Ident, bias=beff_sb[:, ft : ft + 1]
                    )
                return ps

            def emit_proj(concat, y, it2, ft, kts=range(8), ps=None):
                ts = it2 * S
                kts = list(kts)
                if ps is None:
                    ps = psC.tile([P, 512], F32, tag="chain")
                for kt in kts:
                    nc.tensor.matmul(
                        ps[:, 0:S],
                        wpr_sb[:, kt, ft * 128 : (ft + 1) * 128],
                        concat[:, kt, ts : ts + S],
                        start=(kt == 0),
                        stop=(kt == 7),
                    )
                if kts[-1] == 7:
                    nc.scalar.activation(
                        y[:, ft, it2, :], ps[:, 0:S], Ident,
                        bias=beff_sb[:, ft : ft + 1],
                    )
                return ps

            def emit_wave_sc(roped, it2, hp):
                """Scores for head pair hp. Per head one PSUM bank holding
                jt0 at cols 0:S and jt1 at cols 256:256+S. Both writes are
                full overwrites of disjoint ranges (start=True stop=True),
                never accumulation, so sharing the bank is safe on HW (start
                clears has_written bits, not data)."""
                ts = it2 * S
                sc = psS.tile([P, 1024], F32, tag="sc")
                kqs = []
                for bk, h in ((0, 2 * hp), (1, 2 * hp + 1)):
                    hb = 64 * (h % 2)
                    kT = roped[hb : hb + 64, 8 + h // 2, ts : ts + S]
                    qT = roped[hb : hb + 64, h // 2, ts : ts + S]
                    kqs.append((bk, kT, qT))
                # jt0 of both heads first so e0 is ready one matmul earlier
                for bk, kT, qT in kqs:
                    nc.tensor.matmul(
                        sc[:, bk * 512 : bk * 512 + S], kT[:, 0:P], qT,
                        start=True, stop=True, skip_group_check=True,
                    )
                for bk, kT, qT in kqs:
                    nc.tensor.matmul(
                        sc[0:69, bk * 512 + 256 : bk * 512 + 256 + S],
                        kT[:, P:S], qT,
                        start=True, stop=True, skip_group_check=True,
                    )
                sc4 = sc.rearrange("p (b c) -> p b c", b=2)
                e0 = ep.tile([P, 2, S], BF16, tag="e0")
                e1 = ep.tile([P, 2, S], BF16, tag="e1")
                nc.scalar.activation(e0, sc4[:, :, 0:S], Exp, scale=0.125)
                nc.scalar.activation(
                    e1[0:69], sc4[0:69, :, 256 : 256 + S], Exp, scale=0.125
                )
                return e0, e1

            def emit_wave_av(v65, concat, e0, e1, it2, hp):
                """AV + normalization for a head pair; per head one standard
                2-matmul accumulation group in its own bank."""
                ts = it2 * S
                hA, hB = 2 * hp, 2 * hp + 1
                av = psV.tile([P, 1024], F32, tag="av")
                nc.tensor.matmul(
                    av[:, 0:S], v65[it2][0][:, hA, :], e0[:, 0],
                    start=True, stop=False,
                )
                nc.tensor.matmul(
                    av[:, 512 : 512 + S], v65[it2][0][:, hB, :], e0[:, 1],
                    start=True, stop=False,
                )
                nc.tensor.matmul(
                    av[:, 0:S], v65[it2][1][0:69, hA, :], e1[0:69, 0],
                    start=False, stop=True,
                )
                nc.tensor.matmul(
                    av[:, 512 : 512 + S], v65[it2][1][0:69, hB, :], e1[0:69, 1],
                    start=False, stop=True,
                )
                av4 = av.rearrange("p (b c) -> p b c", b=2)
                rb = wk.tile([64, 2, S], F32, tag="rb")
                nc.vector.reciprocal_approx_fast(rb, av4[0:64, :, 0:S])
                nc.vector.tensor_mul(
                    concat[0:64, hp, ts : ts + S], av4[64:128, 0, 0:S], rb[:, 0]
                )
                nc.vector.tensor_mul(
                    concat[64:128, hp, ts : ts + S], av4[64:128, 1, 0:S], rb[:, 1]
                )

            # ---- prologue: pair-0 QKV + V(it0), bare (DMA-paced anyway) ----
            roped_cur = rp.tile([P, 16, W], BF16)
            for ft in range(16):
                emit_qk_ft(x_first, roped_cur, ft)
            v65_cur = [[None, None], [None, None]]
            for tt in range(2):
                v65_cur[0][tt] = make_vt(0, tt)
                for nk in range(2):
                    emit_v_nk(x_first, v65_cur[0][tt], 0, tt, nk)

            # ---- software-pipelined pair phases with a filler-unit queue ----
            queue = deque()
            x_cur = x_first

            def enq_split(fn):
                """Enqueue a chain as two 4-matmul halves sharing one psum
                tile (finer filler granularity for slot-starved phases)."""
                sh = {}

                def a():
                    sh["ps"] = fn(kts=range(0, 4))

                def b():
                    fn(kts=range(4, 8), ps=sh["ps"])

                queue.append(a)
                queue.append(b)

            def enq_v_units(x_sb, v65dst, split):
                for it2 in range(2):
                    for tt in range(2):
                        v65dst[it2][tt] = make_vt(it2, tt)
                        for nk in range(2):
                            fn = (
                                lambda x=x_sb, vt=v65dst[it2][tt], i=it2,
                                t=tt, n=nk, **kw: emit_v_nk(x, vt, i, t, n, **kw)
                            )
                            if split:
                                enq_split(fn)
                            else:
                                queue.append(fn)

            concat_prev = None
            yF_prev = None

            for pr in range(NPAIR):
                nxt = pr + 1 if pr + 1 < NPAIR else None
                concat = cc.tile([P, 8, W], BF16)

                if pr == 0:
                    # v65 for pair-0 it1, consumed by the first it0 pops
                    for tt in range(2):
                        v65_cur[1][tt] = make_vt(1, tt)
                        for nk in range(2):
                            queue.append(
                                (lambda x=x_cur, vt=v65_cur[1][tt], t=tt, n=nk:
                                 emit_v_nk(x, vt, 1, t, n))
                            )

                if nxt is not None:
                    x_nxt = xp.tile([P, 8, W], BF16, tag="x")
                    nc.sync.dma_start(x_nxt, xT[:, nxt])
                    roped_nxt = rp.tile([P, 16, W], BF16)
                    for ft in range(16):
                        queue.append(
                            (lambda x=x_nxt, r=roped_nxt, f=ft:
                             emit_qk_ft(x, r, f))
                        )
                else:
                    # final phase it0 filler: fused proj of the previous pair,
                    # split into half-chains to cover all 16 slots
                    yF = yp.tile([P, 8, W], F32, tag="yF", bufs=1)
                    for ft in range(8):
                        enq_split(
                            lambda c=concat_prev, y=yF, f=ft, **kw:
                            emit_proj_fused(c, y, f, **kw)
                        )
                    yF_prev = yF

                # it0 waves (16 pops, filler units between scores and AV)
                for hp in range(8):
                    e0, e1 = emit_wave_sc(roped_cur, 0, hp)
                    if queue:
                        queue.popleft()()
                    emit_wave_av(v65_cur, concat, e0, e1, 0, hp)
                    if queue:
                        queue.popleft()()

                if nxt is not None:
                    v65_nxt = [[None, None], [None, None]]
                    enq_v_units(x_nxt, v65_nxt, split=(pr == 0))
                    if pr >= 1:
                        # fused proj of pair pr-1 (whole chains)
                        yF = yp.tile([P, 8, W], F32, tag="yF", bufs=1)
                        for ft in range(8):
                            queue.append(
                                (lambda c=concat_prev, y=yF, f=ft:
                                 emit_proj_fused(c, y, f))
                            )
                        yF_prev = yF
                else:
                    v65_nxt = None
                    # deferred pair-2 output is fully emitted by now
                    nc.sync.dma_start(
                        out[:, NPAIR - 2],
                        yF_prev.rearrange("p f (it s) -> p f it s", it=2),
                    )
                    # final phase it1 filler: this pair's it0 proj, split
                    y3 = yp.tile([P, 8, 2, S], F32, tag="y3", bufs=1)
                    for ft in range(8):
                        enq_split(
                            lambda c=concat, y=y3, f=ft, **kw:
                            emit_proj(c, y, 0, f, **kw)
                        )

                # it1 waves (16 pops)
                for hp in range(8):
                    e0, e1 = emit_wave_sc(roped_cur, 1, hp)
                    if queue:
                        queue.popleft()()
                    emit_wave_av(v65_cur, concat, e0, e1, 1, hp)
                    if queue:
                        queue.popleft()()

                # drain leftovers (pure chain work, gap-free)
                while queue:
                    queue.popleft()()

                if debug_taps and pr == 0:
                    nc.sync.dma_start(
                        dbg_roped.rearrange("p (f w) -> p f w", f=16), roped_cur
                    )
                    nc.sync.dma_start(
                        dbg_v.rearrange("p (h d) -> p h d", h=16), v65_cur[0][0]
                    )
                    nc.sync.dma_start(
                        dbg_concat.rearrange("p (j w) -> p j w", j=8), concat
                    )
                    nc.sync.dma_start(dbg_e[:, :], e0[:, 0])

                if nxt is not None:
                    if pr >= 1:
                        # pair pr-1's fused output is complete after the drain
                        nc.sync.dma_start(
                            out[:, pr - 1],
                            yF_prev.rearrange("p f (it s) -> p f it s", it=2),
                        )
                    concat_prev = concat
                    x_cur, roped_cur, v65_cur = x_nxt, roped_nxt, v65_nxt
                else:
                    # tail: this pair's it0 output, then bare it1 proj chains
                    # with the output streamed out in 2-ft chunks so the last
                    # DMA only moves ~200KB after the final IDENT
                    nc.sync.dma_start(out[:, pr, :, 0, :], y3[:, :, 0])
                    for ft in range(8):
                        emit_proj(concat, y3, 1, ft)
                        if ft % 2 == 1:
                            nc.sync.dma_start(
                                out[:, pr, ft - 1 : ft + 1, 1, :],
                                y3[:, ft - 1 : ft + 1, 1],
                            )

    nc.compile()
    return nc


def host_pack_inputs(x, qkv_w, qkv_b, proj_w, proj_b, n_items=NI):
    """Build per-core in_maps (host-side layout only, no math on x)."""
    perm, cosT, sinST, pmatT = _host_tables()
    wqk_dev, wv_dev, wpr_dev, bqk_dev, beff_dev = _pack_weights(
        qkv_w, qkv_b, proj_w, proj_b, perm
    )
    shared = {
        "wqk": np.ascontiguousarray(wqk_dev.reshape(P, -1)),
        "wv": np.ascontiguousarray(wv_dev.reshape(P, -1)),
        "wpr": np.ascontiguousarray(wpr_dev.reshape(P, -1)),
        "pmat": np.ascontiguousarray(pmatT),
        "bqk": bqk_dev,
        "beff": beff_dev,
        "cosT": np.ascontiguousarray(cosT),
        "sinST": np.ascontiguousarray(sinST),
    }
    n_cores = x.shape[0] // n_items
    in_maps = []
    for c in range(n_cores):
        xs = x[c * n_items : (c + 1) * n_items]  # [ni, 197, 1024]
        # device layout [p, pair, kt, w]: feature kt*128+p, token pair*394+w
        xt = xs.reshape(NPAIR, W, C).astype(BF16_NP)
        xt = xt.transpose(2, 0, 1).reshape(8, 128, NPAIR, W).transpose(1, 2, 0, 3)
        in_maps.append(
            {"xT": np.ascontiguousarray(xt.reshape(P, -1)), **shared}
        )
    return in_maps


def unpack_output(results, n_items=NI):
    """results: per-core {'out': [128, NPAIR, 2, 8, S]} -> full (B, N, C)."""
    outs = []
    for r in results:
        yT = r["out"].reshape(P, NPAIR, 8, 2, S)
        # feature o*128+p, token pr*394+it*197+s; device layout [p,pr,o,it,s]
        y = yT.transpose(1, 3, 4, 2, 0).reshape(n_items * S, C)
        outs.append(y.reshape(n_items, S, C))
    return np.concatenate(outs, axis=0)


_CACHED = {}


def kernel(x, qkv_w, qkv_b, proj_w, proj_b):
    from concourse.bass_utils import run_bass_kernel_spmd

    x = np.asarray(x, dtype=np.float32)
    qkv_w = np.asarray(qkv_w, dtype=np.float32)
    qkv_b = np.asarray(qkv_b, dtype=np.float32)
    proj_w = np.asarray(proj_w, dtype=np.float32)
    proj_b = np.asarray(proj_b, dtype=np.float32)

    if "nc" not in _CACHED:
        _CACHED["nc"] = build_nc(NI)
    nc = _CACHED["nc"]
    in_maps = host_pack_inputs(x, qkv_w, qkv_b, proj_w, proj_b, NI)
    res = run_bass_kernel_spmd(nc, in_maps, core_ids=list(range(N_CORES)))
    return unpack_output(res.results, NI).astype(np.float32)


if __name__ == "__main__":
    pass


# revision 51
# speedup vs baseline: 1.1889x; 1.1837x over previous
"""Trainium2 Bass kernel for ViT-style attention block with RoPE.

Problem: x(64,197,1024), qkv(3072x1024)+b, proj(1024x1024)+b, H=16 heads,
RoPE (interleaved pairs, tiled cos/sin tables) on all tokens but CLS.

Strategy: data-parallel over batch across 8 cores (8 items each, no
collectives). Host pre-transposes all operands so the device only runs
matmuls / softmax / RoPE in "transposed" layouts:

  - qk part:  qkT[f, t] = Wqk^T stationary x xT moving   (features on partitions)
  - v part:   v[t, f]   = xT stationary x Wv moving      (tokens on partitions)
  - scores:   scT[j, i] = kT(lhsT) x qT(rhs); per head one PSUM bank holds
              jt0 (cols 0:S) and jt1 (cols 256:256+S) as two non-accumulating
              full-overwrite matmuls (start=True resets has_written bits for
              the bank but not the data, so disjoint overwrites are safe;
              never pack two ACCUMULATING groups in one bank)
  - softmax:  exp on ScalarE (scale=1/8, no max subtraction; |logits|<~5),
              denominators via ones-columns PREPENDED to v (cols 0:64, so the
              sums land at partition base 0: reciprocal_approx_fast silently
              mis-reads partition-offset inputs on HW), normalization =
              reciprocal + DVE mult (standard DVE ops handle offsets fine)
  - AV:       per head a standard 2-matmul accumulation group in its own bank
  - RoPE:     q' = (q+b)*cos + (P(q+b))*sinS where P is a 128x128 block-swap
              permutation done on the TensorEngine; sign and d-permutation
              folded into host-built tables; elementwise split GpSimd/DVE
  - v bias:   folded into proj bias on host (attn rows sum to 1)
  - proj:     yT = Wproj^T stationary x concatT moving, bias on ScalarE;
              pairs 0..2 project both items in one N=394 chain (each weight
              tile loaded once -> half the LDWEIGHTS), shifted one phase later

Scheduling: a software-pipelined unit queue interleaves QKV/V/proj chain
work between each wave's scores and AV matmuls so the PE never waits on
the ScalarE exp; PSUM is exactly 8 banks: chains 2x[P,512] (bufs=3 incl.
spare), rope 1, scores [P,1024], AV [P,1024]. DRAM layouts make every DMA
contiguous per partition row (fewer descriptors; each dma_start costs
~0.6us of serial DIRECT2D descriptor generation on SyncE, so the startup
stream is ordered x0/fg0 first, then consts, then bulk weights).
"""

import sys
from collections import deque

for _p in ("/opt/trn_rl_repo", "/opt/pypackages"):
    if _p not in sys.path:
        sys.path.append(_p)

import numpy as np
import ml_dtypes

import concourse.bass as bass
import concourse.tile as tile
from concourse import bacc
from concourse import mybir

F32 = mybir.dt.float32
BF16 = mybir.dt.bfloat16
BF16_NP = ml_dtypes.bfloat16

# Problem constants (hardcoded per the contract)
B, N, C = 64, 197, 1024
H, D = 16, 64
E = 1  # CLS tokens
THETA = 10000.0
N_CORES = 8
NI = B // N_CORES  # items per core = 8
NT = NI * N  # tokens per core = 1576
S = N  # 197
W = 2 * S  # pair width = 394
NPAIR = NI // 2  # 4
P = 128


def _host_tables():
    """RoPE cos/sin in device layout + permutations, all position-only."""
    seq = (224 // 16) ** 2  # 196
    exp = np.arange(0, D, 2, dtype=np.float64) / -D
    base = THETA**exp  # (32,)
    t = np.arange(seq, dtype=np.float64)
    f0 = np.outer(t, base)  # (196, 32)
    f = np.concatenate([f0, f0], axis=-1)  # (196, 64) "tiled"
    cos_ref = np.cos(f)
    sin_ref = np.sin(f)

    # permutation: new dd<32 -> orig 2dd (x0), new dd>=32 -> orig 2(dd-32)+1 (x1)
    perm = np.empty(D, dtype=np.int64)
    perm[:32] = np.arange(32) * 2
    perm[32:] = np.arange(32) * 2 + 1

    # per-token columns for an item: col 0 = CLS (cos=1, sin=0), cols 1..196 = rope
    cos_item = np.ones((D, S), dtype=np.float64)
    sin_item = np.zeros((D, S), dtype=np.float64)
    cos_item[:, 1:] = cos_ref[:, perm].T
    sin_item[:, 1:] = sin_ref[:, perm].T
    # fold rotate-half signs into sin: rot[dd<32] = -q[dd+32], rot[dd>=32] = +q[dd-32]
    sinS_item = sin_item.copy()
    sinS_item[:32, :] *= -1.0

    # pair-width, replicated for the 2 heads in a 128-partition tile
    cosT = np.tile(cos_item, (2, 2)).astype(BF16_NP)  # [128, 394]
    sinST = np.tile(sinS_item, (2, 2)).astype(BF16_NP)  # [128, 394]

    # 128x128 swap permutation (block swap +-32 within each 64-head-half),
    # already transposed for use as lhsT: rot = P @ q  ->  lhsT = P.T
    Pm = np.zeros((P, P), dtype=np.float32)
    for p in range(P):
        src = 64 * (p // 64) + ((p % 64) + 32) % 64
        Pm[p, src] = 1.0
    pmatT = Pm.T.astype(BF16_NP)  # [K=128, M=128]

    return perm, cosT, sinST, pmatT


def _pack_weights(qkv_w, qkv_b, proj_w, proj_b, perm):
    """Host-side weight packing into device layouts (all numpy, one-time)."""
    # feature permutation for q/k heads: rows of qkv_w within each head
    qk_perm = np.concatenate(
        [h * D + perm for h in range(2 * H)]  # q heads then k heads
    )
    wqk = qkv_w[:2048][qk_perm]  # (2048, 1024) permuted
    bqk = qkv_b[:2048][qk_perm]  # (2048,)
    wv = qkv_w[2048:]  # (1024, 1024)
    bv = qkv_b[2048:]

    wqk_T = np.ascontiguousarray(wqk.T).astype(BF16_NP)  # [1024 k, 2048 f]
    wv_T = np.ascontiguousarray(wv.T).astype(BF16_NP)  # [1024 k, 1024 f]
    proj_wT = np.ascontiguousarray(proj_w.T).astype(BF16_NP)  # [1024, 1024]

    # device layouts with contiguous per-partition DMA blocks:
    # wqk: [p, fg, o, 256] with k = o*128+p, f = fg*256+c
    wqk_dev = np.ascontiguousarray(
        wqk_T.reshape(8, 128, 8, 256).transpose(1, 2, 0, 3)
    )
    # wv/wpr: [p, o, 1024]
    wv_dev = np.ascontiguousarray(wv_T.reshape(8, 128, 1024).transpose(1, 0, 2))
    wpr_dev = np.ascontiguousarray(proj_wT.reshape(8, 128, 1024).transpose(1, 0, 2))

    # biases in [128, ftile] per-partition layout
    bqk_dev = np.ascontiguousarray(bqk.reshape(16, 128).T).astype(np.float32)
    # v bias folded into proj bias: y = concat@W^T + (W@bv + pb)
    beff = proj_w.astype(np.float64) @ bv.astype(np.float64) + proj_b
    beff_dev = np.ascontiguousarray(beff.reshape(8, 128).T).astype(np.float32)
    return wqk_dev, wv_dev, wpr_dev, bqk_dev, beff_dev


def build_nc(n_items=NI, debug_taps=False):
    """Build the per-core Bass graph. SPMD: same graph on all cores."""
    assert n_items == NI
    nc = bacc.Bacc(None, target_bir_lowering=False, debug=False)

    # all DRAM params are 2-D [P, cols]; >2-D shapes are realized as device-
    # side rearranges (the HW ingestion path lays out 2-D exactly like the
    # host's C-order arrays)
    xT2 = nc.declare_dram_parameter("xT", [P, NPAIR * 8 * W], BF16, isOutput=False)
    wqk2 = nc.declare_dram_parameter("wqk", [P, 8 * 8 * 256], BF16, isOutput=False)
    wv2 = nc.declare_dram_parameter("wv", [P, 8 * C], BF16, isOutput=False)
    wpr2 = nc.declare_dram_parameter("wpr", [P, 8 * C], BF16, isOutput=False)
    pmat = nc.declare_dram_parameter("pmat", [P, P], BF16, isOutput=False)
    bqk = nc.declare_dram_parameter("bqk", [P, 16], F32, isOutput=False)
    beff = nc.declare_dram_parameter("beff", [P, 8], F32, isOutput=False)
    cosT = nc.declare_dram_parameter("cosT", [P, W], BF16, isOutput=False)
    sinST = nc.declare_dram_parameter("sinST", [P, W], BF16, isOutput=False)
    out2 = nc.declare_dram_parameter(
        "out", [P, NPAIR * 2 * 8 * S], F32, isOutput=True
    )
    if debug_taps:
        dbg_roped = nc.declare_dram_parameter("dbg_roped", [P, 16 * W], BF16, True)
        dbg_v = nc.declare_dram_parameter("dbg_v", [P, 16 * 128], BF16, True)
        dbg_concat = nc.declare_dram_parameter("dbg_concat", [P, 8 * W], BF16, True)
        dbg_e = nc.declare_dram_parameter("dbg_e", [P, S], BF16, True)
    xT = xT2.rearrange("p (pr k w) -> p pr k w", pr=NPAIR, k=8)
    wqk = wqk2.rearrange("p (fg o c) -> p fg o c", fg=8, o=8)
    wv = wv2.rearrange("p (o f) -> p o f", o=8)
    wpr = wpr2.rearrange("p (o f) -> p o f", o=8)
    out = out2.rearrange("p (pr o it s) -> p pr o it s", pr=NPAIR, o=8, it=2)

    Exp = mybir.ActivationFunctionType.Exp
    Ident = mybir.ActivationFunctionType.Identity

    with tile.TileContext(nc) as tc:
        with (
            tc.tile_pool(name="const", bufs=1) as const,
            tc.tile_pool(name="xp", bufs=2) as xp,
            tc.tile_pool(name="roped", bufs=2) as rp,
            tc.tile_pool(name="vp", bufs=2) as vp,
            tc.tile_pool(name="work", bufs=2) as wk,
            tc.tile_pool(name="ep", bufs=2) as ep,
            tc.tile_pool(name="cc", bufs=2) as cc,
            tc.tile_pool(name="yp", bufs=2) as yp,
            tc.tile_pool(name="psC", bufs=3, space="PSUM") as psC,
            tc.tile_pool(name="psR", bufs=1, space="PSUM") as psR,
            tc.tile_pool(name="psS", bufs=1, space="PSUM") as psS,
            tc.tile_pool(name="psV", bufs=1, space="PSUM") as psV,
        ):
            # ---- early DMAs: the first chain's inputs (x0, fg0) lead, then
            # the consts (needed ~2us later by the first IDENT/rope), then
            # the rest of the wqk stream and v/proj weights ----
            x_first = xp.tile([P, 8, W], BF16, tag="x")
            wqk_sb = const.tile([P, 8, 8, 256], BF16)
            nc.sync.dma_start(x_first, xT[:, 0])
            nc.sync.dma_start(wqk_sb[:, 0], wqk[:, 0])

            bqk_sb = const.tile([P, 16], F32)
            nc.sync.dma_start(bqk_sb, bqk[:, :])
            beff_sb = const.tile([P, 8], F32)
            nc.sync.dma_start(beff_sb, beff[:, :])
            cos_sb = const.tile([P, W], BF16)
            nc.sync.dma_start(cos_sb, cosT[:, :])
            sin_sb = const.tile([P, W], BF16)
            nc.sync.dma_start(sin_sb, sinST[:, :])
            pmat_sb = const.tile([P, P], BF16)
            nc.sync.dma_start(pmat_sb, pmat[:, :])

            # warm the activation table so the 1.3us table load is off the
            # critical path of the first bias-add
            dummy = const.tile([P, 8], F32)
            nc.gpsimd.memset(dummy, 0.0)
            dummy2 = const.tile([P, 8], F32)
            nc.scalar.activation(dummy2, dummy, Exp, scale=0.125)

            for fg in range(1, 8):
                nc.sync.dma_start(wqk_sb[:, fg], wqk[:, fg])
            wv_sb = const.tile([P, 8, C], BF16)
            nc.sync.dma_start(wv_sb, wv[:, :, :])
            wpr_sb = const.tile([P, 8, C], BF16)
            nc.sync.dma_start(wpr_sb, wpr[:, :, :])

            # ---- emission helpers ----
            def emit_qk_ft(x_sb, roped, ft):
                ps = psC.tile([P, 512], F32, tag="chain")
                fg, half = ft // 2, ft % 2
                for kt in range(8):
                    nc.tensor.matmul(
                        ps[:, 0:W],
                        wqk_sb[:, fg, kt, half * 128 : (half + 1) * 128],
                        x_sb[:, kt, :],
                        start=(kt == 0),
                        stop=(kt == 7),
                    )
                tmp = wk.tile([P, W], BF16, tag="tmp")
                nc.scalar.activation(
                    tmp, ps[:, 0:W], Ident, bias=bqk_sb[:, ft : ft + 1]
                )
                psr = psR.tile([P, 512], F32, tag="rope")
                nc.tensor.matmul(psr[:, 0:W], pmat_sb, tmp, start=True, stop=True)
                acc = wk.tile([P, W], BF16, tag="acc")
                nc.gpsimd.tensor_mul(acc, tmp, cos_sb)
                rot2 = wk.tile([P, W], BF16, tag="rot2")
                nc.vector.tensor_mul(rot2, psr[:, 0:W], sin_sb)
                nc.gpsimd.tensor_add(roped[:, ft, :], acc, rot2)

            def make_vt(it2, tt):
                pcount = 128 if tt == 0 else 69
                vt = vp.tile([P, 16, 128], BF16, tag=f"v{it2}{tt}")
                return vt

            def emit_v_nk(x_sb, vt, it2, tt, nk, kts=range(8), ps=None):
                pcount = 128 if tt == 0 else 69
                kts = list(kts)
                if ps is None:
                    ps = psC.tile([P, 512], F32, tag="chain")
                for kt in kts:
                    nc.tensor.matmul(
                        ps[:pcount, :],
                        x_sb[:, kt, it2 * S + tt * P : it2 * S + tt * P + pcount],
                        wv_sb[:, kt, nk * 512 : (nk + 1) * 512],
                        start=(kt == 0),
                        stop=(kt == 7),
                    )
                if kts[-1] != 7:
                    return ps
                # ones in cols 0:64, features in 64:128 so the AV output has
                # the softmax sums at partition base 0 (reciprocal_approx_fast
                # mis-reads partition-offset inputs on HW)
                nc.vector.tensor_copy(
                    vt[:pcount, nk * 8 : (nk + 1) * 8, 64:128],
                    ps[:pcount, :].rearrange("p (h d) -> p h d", d=64),
                )
                if nk == 1:
                    nc.gpsimd.memset(vt[:pcount, :, 0:64], 1.0)

            def emit_proj_fused(concat, y, ft, kts=range(8), ps=None):
                """proj for BOTH items of a pair at once: N=W columns, so
                each weight tile is loaded once for 394 output columns
                (halves LDWEIGHTS traffic vs per-item chains)."""
                kts = list(kts)
                if ps is None:
                    ps = psC.tile([P, 512], F32, tag="chain")
                for kt in kts:
                    nc.tensor.matmul(
                        ps[:, 0:W],
                        wpr_sb[:, kt, ft * 128 : (ft + 1) * 128],
                        concat[:, kt, :],
                        start=(kt == 0),
                        stop=(kt == 7),
                    )
                if kts[-1] == 7:
                    nc.scalar.activation(
                        y[:, ft, :], ps[:, 0:W], Ident, bias=beff_sb[:, ft : ft + 1]
                    )
                return ps

            def emit_proj(concat, y, it2, ft, kts=range(8), ps=None):
                ts = it2 * S
                kts = list(kts)
                if ps is None:
                    ps = psC.tile([P, 512], F32, tag="chain")
                for kt in kts:
                    nc.tensor.matmul(
                        ps[:, 0:S],
                        wpr_sb[:, kt, ft * 128 : (ft + 1) * 128],
                        concat[:, kt, ts : ts + S],
                        start=(kt == 0),
                        stop=(kt == 7),
                    )
                if kts[-1] == 7:
                    nc.scalar.activation(
                        y[:, ft, it2, :], ps[:, 0:S], Ident,
                        bias=beff_sb[:, ft : ft + 1],
                    )
                return ps

            def emit_wave_sc(roped, it2, hp):
                """Scores for head pair hp. Per head one PSUM bank holding
                jt0 at cols 0:S and jt1 at cols 256:256+S. Both writes are
                full overwrites of disjoint ranges (start=True stop=True),
                never accumulation, so sharing the bank is safe on HW (start
                clears has_written bits, not data)."""
                ts = it2 * S
                sc = psS.tile([P, 1024], F32, tag="sc")
                kqs = []
                for bk, h in ((0, 2 * hp), (1, 2 * hp + 1)):
                    hb = 64 * (h % 2)
                    kT = roped[hb : hb + 64, 8 + h // 2, ts : ts + S]
                    qT = roped[hb : hb + 64, h // 2, ts : ts + S]
                    kqs.append((bk, kT, qT))
                # jt0 of both heads first so e0 is ready one matmul earlier
                for bk, kT, qT in kqs:
                    nc.tensor.matmul(
                        sc[:, bk * 512 : bk * 512 + S], kT[:, 0:P], qT,
                        start=True, stop=True, skip_group_check=True,
                    )
                for bk, kT, qT in kqs:
                    nc.tensor.matmul(
                        sc[0:69, bk * 512 + 256 : bk * 512 + 256 + S],
                        kT[:, P:S], qT,
                        start=True, stop=True, skip_group_check=True,
                    )
                sc4 = sc.rearrange("p (b c) -> p b c", b=2)
                e0 = ep.tile([P, 2, S], BF16, tag="e0")
                e1 = ep.tile([P, 2, S], BF16, tag="e1")
                nc.scalar.activation(e0, sc4[:, :, 0:S], Exp, scale=0.125)
                nc.scalar.activation(
                    e1[0:69], sc4[0:69, :, 256 : 256 + S], Exp, scale=0.125
                )
                return e0, e1

            def emit_wave_av(v65, concat, e0, e1, it2, hp):
                """AV + normalization for a head pair; per head one standard
                2-matmul accumulation group in its own bank."""
                ts = it2 * S
                hA, hB = 2 * hp, 2 * hp + 1
                av = psV.tile([P, 1024], F32, tag="av")
                nc.tensor.matmul(
                    av[:, 0:S], v65[it2][0][:, hA, :], e0[:, 0],
                    start=True, stop=False,
                )
                nc.tensor.matmul(
                    av[:, 512 : 512 + S], v65[it2][0][:, hB, :], e0[:, 1],
                    start=True, stop=False,
                )
                nc.tensor.matmul(
                    av[:, 0:S], v65[it2][1][0:69, hA, :], e1[0:69, 0],
                    start=False, stop=True,
                )
                nc.tensor.matmul(
                    av[:, 512 : 512 + S], v65[it2][1][0:69, hB, :], e1[0:69, 1],
                    start=False, stop=True,
                )
                av4 = av.rearrange("p (b c) -> p b c", b=2)
                rb = wk.tile([64, 2, S], F32, tag="rb")
                nc.vector.reciprocal_approx_fast(rb, av4[0:64, :, 0:S])
                nc.vector.tensor_mul(
                    concat[0:64, hp, ts : ts + S], av4[64:128, 0, 0:S], rb[:, 0]
                )
                nc.vector.tensor_mul(
                    concat[64:128, hp, ts : ts + S], av4[64:128, 1, 0:S], rb[:, 1]
                )

            # ---- prologue: pair-0 QKV + V(it0), bare (DMA-paced anyway) ----
            roped_cur = rp.tile([P, 16, W], BF16)
            for ft in range(16):
                emit_qk_ft(x_first, roped_cur, ft)
            v65_cur = [[None, None], [None, None]]
            for tt in range(2):
                v65_cur[0][tt] = make_vt(0, tt)
                for nk in range(2):
                    emit_v_nk(x_first, v65_cur[0][tt], 0, tt, nk)

            # ---- software-pipelined pair phases with a filler-unit queue ----
            queue = deque()
            x_cur = x_first

            def enq_split(fn):
                """Enqueue a chain as two 4-matmul halves sharing one psum
                tile (finer filler granularity for slot-starved phases)."""
                sh = {}

                def a():
                    sh["ps"] = fn(kts=range(0, 4))

                def b():
                    fn(kts=range(4, 8), ps=sh["ps"])

                queue.append(a)
                queue.append(b)

            def enq_v_units(x_sb, v65dst, split):
                for it2 in range(2):
                    for tt in range(2):
                        v65dst[it2][tt] = make_vt(it2, tt)
                        for nk in range(2):
                            fn = (
                                lambda x=x_sb, vt=v65dst[it2][tt], i=it2,
                                t=tt, n=nk, **kw: emit_v_nk(x, vt, i, t, n, **kw)
                            )
                            if split:
                                enq_split(fn)
                            else:
                                queue.append(fn)

            concat_prev = None
            yF_prev = None

            for pr in range(NPAIR):
                nxt = pr + 1 if pr + 1 < NPAIR else None
                concat = cc.tile([P, 8, W], BF16)

                if pr == 0:
                    # v65 for pair-0 it1, consumed by the first it0 pops
                    for tt in range(2):
                        v65_cur[1][tt] = make_vt(1, tt)
                        for nk in range(2):
                            queue.append(
                                (lambda x=x_cur, vt=v65_cur[1][tt], t=tt, n=nk:
                                 emit_v_nk(x, vt, 1, t, n))
                            )

                if nxt is not None:
                    x_nxt = xp.tile([P, 8, W], BF16, tag="x")
                    nc.sync.dma_start(x_nxt, xT[:, nxt])
                    roped_nxt = rp.tile([P, 16, W], BF16)
                    for ft in range(16):
                        queue.append(
                            (lambda x=x_nxt, r=roped_nxt, f=ft:
                             emit_qk_ft(x, r, f))
                        )
                else:
                    # final phase it0 filler: fused proj of the previous pair,
                    # split into half-chains to cover all 16 slots
                    yF = yp.tile([P, 8, W], F32, tag="yF", bufs=1)
                    for ft in range(8):
                        enq_split(
                            lambda c=concat_prev, y=yF, f=ft, **kw:
                            emit_proj_fused(c, y, f, **kw)
                        )
                    yF_prev = yF

                # it0 waves (16 pops, filler units between scores and AV)
                for hp in range(8):
                    e0, e1 = emit_wave_sc(roped_cur, 0, hp)
                    if queue:
                        queue.popleft()()
                    emit_wave_av(v65_cur, concat, e0, e1, 0, hp)
                    if queue:
                        queue.popleft()()

                if nxt is not None:
                    v65_nxt = [[None, None], [None, None]]
                    enq_v_units(x_nxt, v65_nxt, split=(pr == 0))
                    if pr >= 1:
                        # fused proj of pair pr-1 (whole chains)
                        yF = yp.tile([P, 8, W], F32, tag="yF", bufs=1)
                        for ft in range(8):
                            queue.append(
                                (lambda c=concat_prev, y=yF, f=ft:
                                 emit_proj_fused(c, y, f))
                            )
                        yF_prev = yF
                else:
                    v65_nxt = None
                    # deferred pair-2 output is fully emitted by now
                    nc.sync.dma_start(
                        out[:, NPAIR - 2],
                        yF_prev.rearrange("p f (it s) -> p f it s", it=2),
                    )
                    # final phase it1 filler: this pair's it0 proj, split
                    y3 = yp.tile([P, 8, 2, S], F32, tag="y3", bufs=1)
                    for ft in range(8):
                        enq_split(
                            lambda c=concat, y=y3, f=ft, **kw:
                            emit_proj(c, y, 0, f, **kw)
                        )

                # it1 waves (16 pops)
                for hp in range(8):
                    e0, e1 = emit_wave_sc(roped_cur, 1, hp)
                    if queue:
                        queue.popleft()()
                    emit_wave_av(v65_cur, concat, e0, e1, 1, hp)
                    if queue:
                        queue.popleft()()

                # drain leftovers (pure chain work, gap-free)
                while queue:
                    queue.popleft()()

                if debug_taps and pr == 0:
                    nc.sync.dma_start(
                        dbg_roped.rearrange("p (f w) -> p f w", f=16), roped_cur
                    )
                    nc.sync.dma_start(
                        dbg_v.rearrange("p (h d) -> p h d", h=16), v65_cur[0][0]
                    )
                    nc.sync.dma_start(
                        dbg_concat.rearrange("p (j w) -> p j w", j=8), concat
                    )
                    nc.sync.dma_start(dbg_e[:, :], e0[:, 0])

                if nxt is not None:
                    if pr >= 1:
                        # pair pr-1's fused output is complete after the drain
                        nc.sync.dma_start(
                            out[:, pr - 1],
                            yF_prev.rearrange("p f (it s) -> p f it s", it=2),
                        )
                    concat_prev = concat
                    x_cur, roped_cur, v65_cur = x_nxt, roped_nxt, v65_nxt
                else:
                    # tail: this pair's it0 output, then bare it1 proj chains
                    # with the output streamed out in 2-ft chunks so the last
                    # DMA only moves ~200KB after the final IDENT
                    nc.sync.dma_start(out[:, pr, :, 0, :], y3[:, :, 0])
                    for ft in range(8):
                        emit_proj(concat, y3, 1, ft)
                        if ft % 2 == 1:
                            nc.sync.dma_start(
                                out[:, pr, ft - 1 : ft + 1, 1, :],
                                y3[:, ft - 1 : ft + 1, 1],
                            )

    nc.compile()
    return nc


def host_pack_inputs(x, qkv_w, qkv_b, proj_w, proj_b, n_items=NI):
    """Build per-core in_maps (host-side layout only, no math on x)."""
    perm, cosT, sinST, pmatT = _host_tables()
    wqk_dev, wv_dev, wpr_dev, bqk_dev, beff_dev = _pack_weights(
        qkv_w, qkv_b, proj_w, proj_b, perm
    )
    shared = {
        "wqk": np.ascontiguousarray(wqk_dev.reshape(P, -1)),
        "wv": np.ascontiguousarray(wv_dev.reshape(P, -1)),
        "wpr": np.ascontiguousarray(wpr_dev.reshape(P, -1)),
        "pmat": np.ascontiguousarray(pmatT),
        "bqk": bqk_dev,
        "beff": beff_dev,
        "cosT": np.ascontiguousarray(cosT),
        "sinST": np.ascontiguousarray(sinST),
    }
    n_cores = x.shape[0] // n_items
    in_maps = []
    for c in range(n_cores):
        xs = x[c * n_items : (c + 1) * n_items]  # [ni, 197, 1024]
        # device layout [p, pair, kt, w]: feature kt*128+p, token pair*394+w
        xt = xs.reshape(NPAIR, W, C).astype(BF16_NP)
        xt = xt.transpose(2, 0, 1).reshape(8, 128, NPAIR, W).transpose(1, 2, 0, 3)
        in_maps.append(
            {"xT": np.ascontiguousarray(xt.reshape(P, -1)), **shared}
        )
    return in_maps


def unpack_output(results, n_items=NI):
    """results: per-core {'out': [128, NPAIR, 2, 8, S]} -> full (B, N, C)."""
    outs = []
    for r in results:
        yT = r["out"].reshape(P, NPAIR, 8, 2, S)
        # feature o*128+p, token pr*394+it*197+s; device layout [p,pr,o,it,s]
        y = yT.transpose(1, 3, 4, 2, 0).reshape(n_items * S, C)
        outs.append(y.reshape(n_items, S, C))
    return np.concatenate(outs, axis=0)


_CACHED = {}


def kernel(x, qkv_w, qkv_b, proj_w, proj_b):
    from concourse.bass_utils import run_bass_kernel_spmd

    x = np.asarray(x, dtype=np.float32)
    qkv_w = np.asarray(qkv_w, dtype=np.float32)
    qkv_b = np.asarray(qkv_b, dtype=np.float32)
    proj_w = np.asarray(proj_w, dtype=np.float32)
    proj_b = np.asarray(proj_b, dtype=np.float32)

    if "nc" not in _CACHED:
        _CACHED["nc"] = build_nc(NI)
    nc = _CACHED["nc"]
    in_maps = host_pack_inputs(x, qkv_w, qkv_b, proj_w, proj_b, NI)
    res = run_bass_kernel_spmd(nc, in_maps, core_ids=list(range(N_CORES)))
    return unpack_output(res.results, NI).astype(np.float32)


if __name__ == "__main__":
    pass


# revision 53
# speedup vs baseline: 1.1918x; 1.0024x over previous
"""Trainium2 Bass kernel for ViT-style attention block with RoPE.

Problem: x(64,197,1024), qkv(3072x1024)+b, proj(1024x1024)+b, H=16 heads,
RoPE (interleaved pairs, tiled cos/sin tables) on all tokens but CLS.

Strategy: data-parallel over batch across 8 cores (8 items each, no
collectives). Host pre-transposes all operands so the device only runs
matmuls / softmax / RoPE in "transposed" layouts:

  - qk part:  qkT[f, t] = Wqk^T stationary x xT moving   (features on partitions)
  - v part:   v[t, f]   = xT stationary x Wv moving      (tokens on partitions)
  - scores:   scT[j, i] = kT(lhsT) x qT(rhs); per head one PSUM bank holds
              jt0 (cols 0:S) and jt1 (cols 256:256+S) as two non-accumulating
              full-overwrite matmuls (start=True resets has_written bits for
              the bank but not the data, so disjoint overwrites are safe;
              never pack two ACCUMULATING groups in one bank)
  - softmax:  exp on ScalarE (scale=1/8, no max subtraction; |logits|<~5),
              denominators via ones-columns PREPENDED to v (cols 0:64, so the
              sums land at partition base 0: reciprocal_approx_fast silently
              mis-reads partition-offset inputs on HW), normalization =
              reciprocal + DVE mult (standard DVE ops handle offsets fine)
  - AV:       per head a standard 2-matmul accumulation group in its own bank
  - RoPE:     q' = (q+b)*cos + (P(q+b))*sinS where P is a 128x128 block-swap
              permutation done on the TensorEngine; sign and d-permutation
              folded into host-built tables; elementwise split GpSimd/DVE
  - v bias:   folded into proj bias on host (attn rows sum to 1)
  - proj:     yT = Wproj^T stationary x concatT moving, bias on ScalarE;
              pairs 0..2 project both items in one N=394 chain (each weight
              tile loaded once -> half the LDWEIGHTS), shifted one phase later

Scheduling: a software-pipelined unit queue interleaves QKV/V/proj chain
work between each wave's scores and AV matmuls so the PE never waits on
the ScalarE exp; PSUM is exactly 8 banks: chains 2x[P,512] (bufs=3 incl.
spare), rope 1, scores [P,1024], AV [P,1024]. DRAM layouts make every DMA
contiguous per partition row (fewer descriptors; each dma_start costs
~0.6us of serial DIRECT2D descriptor generation on SyncE, so the startup
stream is ordered x0/fg0 first, then consts, then bulk weights).
"""

import sys
from collections import deque

for _p in ("/opt/trn_rl_repo", "/opt/pypackages"):
    if _p not in sys.path:
        sys.path.append(_p)

import numpy as np
import ml_dtypes

import concourse.bass as bass
import concourse.tile as tile
from concourse import bacc
from concourse import mybir

F32 = mybir.dt.float32
BF16 = mybir.dt.bfloat16
BF16_NP = ml_dtypes.bfloat16

# Problem constants (hardcoded per the contract)
B, N, C = 64, 197, 1024
H, D = 16, 64
E = 1  # CLS tokens
THETA = 10000.0
N_CORES = 8
NI = B // N_CORES  # items per core = 8
NT = NI * N  # tokens per core = 1576
S = N  # 197
W = 2 * S  # pair width = 394
NPAIR = NI // 2  # 4
P = 128


def _host_tables():
    """RoPE cos/sin in device layout + permutations, all position-only."""
    seq = (224 // 16) ** 2  # 196
    exp = np.arange(0, D, 2, dtype=np.float64) / -D
    base = THETA**exp  # (32,)
    t = np.arange(seq, dtype=np.float64)
    f0 = np.outer(t, base)  # (196, 32)
    f = np.concatenate([f0, f0], axis=-1)  # (196, 64) "tiled"
    cos_ref = np.cos(f)
    sin_ref = np.sin(f)

    # permutation: new dd<32 -> orig 2dd (x0), new dd>=32 -> orig 2(dd-32)+1 (x1)
    perm = np.empty(D, dtype=np.int64)
    perm[:32] = np.arange(32) * 2
    perm[32:] = np.arange(32) * 2 + 1

    # per-token columns for an item: col 0 = CLS (cos=1, sin=0), cols 1..196 = rope
    cos_item = np.ones((D, S), dtype=np.float64)
    sin_item = np.zeros((D, S), dtype=np.float64)
    cos_item[:, 1:] = cos_ref[:, perm].T
    sin_item[:, 1:] = sin_ref[:, perm].T
    # fold rotate-half signs into sin: rot[dd<32] = -q[dd+32], rot[dd>=32] = +q[dd-32]
    sinS_item = sin_item.copy()
    sinS_item[:32, :] *= -1.0

    # pair-width, replicated for the 2 heads in a 128-partition tile
    cosT = np.tile(cos_item, (2, 2)).astype(BF16_NP)  # [128, 394]
    sinST = np.tile(sinS_item, (2, 2)).astype(BF16_NP)  # [128, 394]

    # 128x128 swap permutation (block swap +-32 within each 64-head-half),
    # already transposed for use as lhsT: rot = P @ q  ->  lhsT = P.T
    Pm = np.zeros((P, P), dtype=np.float32)
    for p in range(P):
        src = 64 * (p // 64) + ((p % 64) + 32) % 64
        Pm[p, src] = 1.0
    pmatT = Pm.T.astype(BF16_NP)  # [K=128, M=128]

    return perm, cosT, sinST, pmatT


def _pack_weights(qkv_w, qkv_b, proj_w, proj_b, perm):
    """Host-side weight packing into device layouts (all numpy, one-time)."""
    # feature permutation for q/k heads: rows of qkv_w within each head
    qk_perm = np.concatenate(
        [h * D + perm for h in range(2 * H)]  # q heads then k heads
    )
    wqk = qkv_w[:2048][qk_perm]  # (2048, 1024) permuted
    bqk = qkv_b[:2048][qk_perm]  # (2048,)
    wv = qkv_w[2048:]  # (1024, 1024)
    bv = qkv_b[2048:]

    wqk_T = np.ascontiguousarray(wqk.T).astype(BF16_NP)  # [1024 k, 2048 f]
    wv_T = np.ascontiguousarray(wv.T).astype(BF16_NP)  # [1024 k, 1024 f]
    proj_wT = np.ascontiguousarray(proj_w.T).astype(BF16_NP)  # [1024, 1024]

    # device layouts with contiguous per-partition DMA blocks:
    # wqk: [p, fg, o, 256] with k = o*128+p, f = fg*256+c
    wqk_dev = np.ascontiguousarray(
        wqk_T.reshape(8, 128, 8, 256).transpose(1, 2, 0, 3)
    )
    # wv/wpr: [p, o, 1024]
    wv_dev = np.ascontiguousarray(wv_T.reshape(8, 128, 1024).transpose(1, 0, 2))
    wpr_dev = np.ascontiguousarray(proj_wT.reshape(8, 128, 1024).transpose(1, 0, 2))

    # biases in [128, ftile] per-partition layout
    bqk_dev = np.ascontiguousarray(bqk.reshape(16, 128).T).astype(np.float32)
    # v bias folded into proj bias: y = concat@W^T + (W@bv + pb)
    beff = proj_w.astype(np.float64) @ bv.astype(np.float64) + proj_b
    beff_dev = np.ascontiguousarray(beff.reshape(8, 128).T).astype(np.float32)
    return wqk_dev, wv_dev, wpr_dev, bqk_dev, beff_dev


def build_nc(n_items=NI, debug_taps=False):
    """Build the per-core Bass graph. SPMD: same graph on all cores."""
    assert n_items == NI
    nc = bacc.Bacc(None, target_bir_lowering=False, debug=False)

    # all DRAM params are 2-D [P, cols]; >2-D shapes are realized as device-
    # side rearranges (the HW ingestion path lays out 2-D exactly like the
    # host's C-order arrays)
    xT2 = nc.declare_dram_parameter("xT", [P, NPAIR * 8 * W], BF16, isOutput=False)
    wqk2 = nc.declare_dram_parameter("wqk", [P, 8 * 8 * 256], BF16, isOutput=False)
    wv2 = nc.declare_dram_parameter("wv", [P, 8 * C], BF16, isOutput=False)
    wpr2 = nc.declare_dram_parameter("wpr", [P, 8 * C], BF16, isOutput=False)
    pmat = nc.declare_dram_parameter("pmat", [P, P], BF16, isOutput=False)
    bqk = nc.declare_dram_parameter("bqk", [P, 16], F32, isOutput=False)
    beff = nc.declare_dram_parameter("beff", [P, 8], F32, isOutput=False)
    cosT = nc.declare_dram_parameter("cosT", [P, W], BF16, isOutput=False)
    sinST = nc.declare_dram_parameter("sinST", [P, W], BF16, isOutput=False)
    out2 = nc.declare_dram_parameter(
        "out", [P, NPAIR * 2 * 8 * S], F32, isOutput=True
    )
    if debug_taps:
        dbg_roped = nc.declare_dram_parameter("dbg_roped", [P, 16 * W], BF16, True)
        dbg_v = nc.declare_dram_parameter("dbg_v", [P, 16 * 128], BF16, True)
        dbg_concat = nc.declare_dram_parameter("dbg_concat", [P, 8 * W], BF16, True)
        dbg_e = nc.declare_dram_parameter("dbg_e", [P, S], BF16, True)
    xT = xT2.rearrange("p (pr k w) -> p pr k w", pr=NPAIR, k=8)
    wqk = wqk2.rearrange("p (fg o c) -> p fg o c", fg=8, o=8)
    wv = wv2.rearrange("p (o f) -> p o f", o=8)
    wpr = wpr2.rearrange("p (o f) -> p o f", o=8)
    out = out2.rearrange("p (pr o it s) -> p pr o it s", pr=NPAIR, o=8, it=2)

    Exp = mybir.ActivationFunctionType.Exp
    Ident = mybir.ActivationFunctionType.Identity

    with tile.TileContext(nc) as tc:
        with (
            tc.tile_pool(name="const", bufs=1) as const,
            tc.tile_pool(name="xp", bufs=2) as xp,
            tc.tile_pool(name="roped", bufs=2) as rp,
            tc.tile_pool(name="vp", bufs=2) as vp,
            tc.tile_pool(name="work", bufs=2) as wk,
            tc.tile_pool(name="ep", bufs=2) as ep,
            tc.tile_pool(name="cc", bufs=2) as cc,
            tc.tile_pool(name="yp", bufs=2) as yp,
            tc.tile_pool(name="psC", bufs=3, space="PSUM") as psC,
            tc.tile_pool(name="psR", bufs=1, space="PSUM") as psR,
            tc.tile_pool(name="psS", bufs=1, space="PSUM") as psS,
            tc.tile_pool(name="psV", bufs=1, space="PSUM") as psV,
        ):
            # ---- early DMAs: the first chain's inputs (x0, fg0) lead, then
            # the consts (needed ~2us later by the first IDENT/rope), then
            # the rest of the wqk stream and v/proj weights ----
            x_first = xp.tile([P, 8, W], BF16, tag="x")
            wqk_sb = const.tile([P, 8, 8, 256], BF16)
            nc.sync.dma_start(x_first, xT[:, 0])
            nc.sync.dma_start(wqk_sb[:, 0], wqk[:, 0])

            bqk_sb = const.tile([P, 16], F32)
            nc.sync.dma_start(bqk_sb, bqk[:, :])
            beff_sb = const.tile([P, 8], F32)
            nc.sync.dma_start(beff_sb, beff[:, :])
            cos_sb = const.tile([P, W], BF16)
            nc.sync.dma_start(cos_sb, cosT[:, :])
            sin_sb = const.tile([P, W], BF16)
            nc.sync.dma_start(sin_sb, sinST[:, :])
            pmat_sb = const.tile([P, P], BF16)
            nc.sync.dma_start(pmat_sb, pmat[:, :])

            # warm the activation table so the 1.3us table load is off the
            # critical path of the first bias-add
            dummy = const.tile([P, 8], F32)
            nc.gpsimd.memset(dummy, 0.0)
            dummy2 = const.tile([P, 8], F32)
            nc.scalar.activation(dummy2, dummy, Exp, scale=0.125)

            for fg in range(1, 8):
                nc.sync.dma_start(wqk_sb[:, fg], wqk[:, fg])
            wv_sb = const.tile([P, 8, C], BF16)
            nc.sync.dma_start(wv_sb, wv[:, :, :])
            wpr_sb = const.tile([P, 8, C], BF16)
            nc.sync.dma_start(wpr_sb, wpr[:, :, :])

            # ---- emission helpers ----
            def emit_qk_ft(x_sb, roped, ft):
                ps = psC.tile([P, 512], F32, tag="chain")
                fg, half = ft // 2, ft % 2
                for kt in range(8):
                    nc.tensor.matmul(
                        ps[:, 0:W],
                        wqk_sb[:, fg, kt, half * 128 : (half + 1) * 128],
                        x_sb[:, kt, :],
                        start=(kt == 0),
                        stop=(kt == 7),
                    )
                tmp = wk.tile([P, W], BF16, tag="tmp")
                nc.scalar.activation(
                    tmp, ps[:, 0:W], Ident, bias=bqk_sb[:, ft : ft + 1]
                )
                psr = psR.tile([P, 512], F32, tag="rope")
                nc.tensor.matmul(psr[:, 0:W], pmat_sb, tmp, start=True, stop=True)
                acc = wk.tile([P, W], BF16, tag="acc")
                nc.gpsimd.tensor_mul(acc, tmp, cos_sb)
                rot2 = wk.tile([P, W], BF16, tag="rot2")
                nc.vector.tensor_mul(rot2, psr[:, 0:W], sin_sb)
                nc.gpsimd.tensor_add(roped[:, ft, :], acc, rot2)

            def make_vt(it2, tt):
                pcount = 128 if tt == 0 else 69
                vt = vp.tile([P, 16, 128], BF16, tag=f"v{it2}{tt}")
                return vt

            def emit_v_nk(x_sb, vt, it2, tt, nk, kts=range(8), ps=None):
                pcount = 128 if tt == 0 else 69
                kts = list(kts)
                if ps is None:
                    ps = psC.tile([P, 512], F32, tag="chain")
                for kt in kts:
                    nc.tensor.matmul(
                        ps[:pcount, :],
                        x_sb[:, kt, it2 * S + tt * P : it2 * S + tt * P + pcount],
                        wv_sb[:, kt, nk * 512 : (nk + 1) * 512],
                        start=(kt == 0),
                        stop=(kt == 7),
                    )
                if kts[-1] != 7:
                    return ps
                # ones in cols 0:64, features in 64:128 so the AV output has
                # the softmax sums at partition base 0 (reciprocal_approx_fast
                # mis-reads partition-offset inputs on HW)
                nc.vector.tensor_copy(
                    vt[:pcount, nk * 8 : (nk + 1) * 8, 64:128],
                    ps[:pcount, :].rearrange("p (h d) -> p h d", d=64),
                )
                if nk == 1:
                    nc.gpsimd.memset(vt[:pcount, :, 0:64], 1.0)

            def emit_proj_fused(concat, y, ft, kts=range(8), ps=None):
                """proj for BOTH items of a pair at once: N=W columns, so
                each weight tile is loaded once for 394 output columns
                (halves LDWEIGHTS traffic vs per-item chains)."""
                kts = list(kts)
                if ps is None:
                    ps = psC.tile([P, 512], F32, tag="chain")
                for kt in kts:
                    nc.tensor.matmul(
                        ps[:, 0:W],
                        wpr_sb[:, kt, ft * 128 : (ft + 1) * 128],
                        concat[:, kt, :],
                        start=(kt == 0),
                        stop=(kt == 7),
                    )
                if kts[-1] == 7:
                    nc.scalar.activation(
                        y[:, ft, :], ps[:, 0:W], Ident, bias=beff_sb[:, ft : ft + 1]
                    )
                return ps

            def emit_proj(concat, y, it2, ft, kts=range(8), ps=None):
                ts = it2 * S
                kts = list(kts)
                if ps is None:
                    ps = psC.tile([P, 512], F32, tag="chain")
                for kt in kts:
                    nc.tensor.matmul(
                        ps[:, 0:S],
                        wpr_sb[:, kt, ft * 128 : (ft + 1) * 128],
                        concat[:, kt, ts : ts + S],
                        start=(kt == 0),
                        stop=(kt == 7),
                    )
                if kts[-1] == 7:
                    nc.scalar.activation(
                        y[:, ft, it2, :], ps[:, 0:S], Ident,
                        bias=beff_sb[:, ft : ft + 1],
                    )
                return ps

            def emit_wave_sc(roped, it2, hp):
                """Scores for head pair hp. Per head one PSUM bank holding
                jt0 at cols 0:S and jt1 at cols 256:256+S. Both writes are
                full overwrites of disjoint ranges (start=True stop=True),
                never accumulation, so sharing the bank is safe on HW (start
                clears has_written bits, not data)."""
                ts = it2 * S
                sc = psS.tile([P, 1024], F32, tag="sc")
                kqs = []
                for bk, h in ((0, 2 * hp), (1, 2 * hp + 1)):
                    hb = 64 * (h % 2)
                    kT = roped[hb : hb + 64, 8 + h // 2, ts : ts + S]
                    qT = roped[hb : hb + 64, h // 2, ts : ts + S]
                    kqs.append((bk, kT, qT))
                # jt0 of both heads first so e0 is ready one matmul earlier
                for bk, kT, qT in kqs:
                    nc.tensor.matmul(
                        sc[:, bk * 512 : bk * 512 + S], kT[:, 0:P], qT,
                        start=True, stop=True, skip_group_check=True,
                    )
                for bk, kT, qT in kqs:
                    nc.tensor.matmul(
                        sc[0:69, bk * 512 + 256 : bk * 512 + 256 + S],
                        kT[:, P:S], qT,
                        start=True, stop=True, skip_group_check=True,
                    )
                sc4 = sc.rearrange("p (b c) -> p b c", b=2)
                e0 = ep.tile([P, 2, S], BF16, tag="e0")
                e1 = ep.tile([P, 2, S], BF16, tag="e1")
                nc.scalar.activation(e0, sc4[:, :, 0:S], Exp, scale=0.125)
                nc.scalar.activation(
                    e1[0:69], sc4[0:69, :, 256 : 256 + S], Exp, scale=0.125
                )
                return e0, e1

            def emit_wave_av(v65, concat, e0, e1, it2, hp):
                """AV + normalization for a head pair; per head one standard
                2-matmul accumulation group in its own bank."""
                ts = it2 * S
                hA, hB = 2 * hp, 2 * hp + 1
                av = psV.tile([P, 1024], F32, tag="av")
                nc.tensor.matmul(
                    av[:, 0:S], v65[it2][0][:, hA, :], e0[:, 0],
                    start=True, stop=False,
                )
                nc.tensor.matmul(
                    av[:, 512 : 512 + S], v65[it2][0][:, hB, :], e0[:, 1],
                    start=True, stop=False,
                )
                nc.tensor.matmul(
                    av[:, 0:S], v65[it2][1][0:69, hA, :], e1[0:69, 0],
                    start=False, stop=True,
                )
                nc.tensor.matmul(
                    av[:, 512 : 512 + S], v65[it2][1][0:69, hB, :], e1[0:69, 1],
                    start=False, stop=True,
                )
                av4 = av.rearrange("p (b c) -> p b c", b=2)
                rb = wk.tile([64, 2, S], F32, tag="rb")
                nc.vector.reciprocal_approx_fast(rb, av4[0:64, :, 0:S])
                nc.vector.tensor_mul(
                    concat[0:64, hp, ts : ts + S], av4[64:128, 0, 0:S], rb[:, 0]
                )
                nc.vector.tensor_mul(
                    concat[64:128, hp, ts : ts + S], av4[64:128, 1, 0:S], rb[:, 1]
                )

            # ---- prologue: pair-0 QKV + V(it0), bare (DMA-paced anyway) ----
            roped_cur = rp.tile([P, 16, W], BF16)
            for ft in range(16):
                emit_qk_ft(x_first, roped_cur, ft)
            v65_cur = [[None, None], [None, None]]
            for tt in range(2):
                v65_cur[0][tt] = make_vt(0, tt)
                for nk in range(2):
                    emit_v_nk(x_first, v65_cur[0][tt], 0, tt, nk)

            # ---- software-pipelined pair phases with a filler-unit queue ----
            queue = deque()
            x_cur = x_first

            def enq_split(fn):
                """Enqueue a chain as two 4-matmul halves sharing one psum
                tile (finer filler granularity for slot-starved phases)."""
                sh = {}

                def a():
                    sh["ps"] = fn(kts=range(0, 4))

                def b():
                    fn(kts=range(4, 8), ps=sh["ps"])

                queue.append(a)
                queue.append(b)

            def enq_v_units(x_sb, v65dst, split):
                for it2 in range(2):
                    for tt in range(2):
                        v65dst[it2][tt] = make_vt(it2, tt)
                        for nk in range(2):
                            fn = (
                                lambda x=x_sb, vt=v65dst[it2][tt], i=it2,
                                t=tt, n=nk, **kw: emit_v_nk(x, vt, i, t, n, **kw)
                            )
                            if split:
                                enq_split(fn)
                            else:
                                queue.append(fn)

            concat_prev = None
            yF_prev = None

            for pr in range(NPAIR):
                nxt = pr + 1 if pr + 1 < NPAIR else None
                concat = cc.tile([P, 8, W], BF16)

                if pr == 0:
                    # v65 for pair-0 it1, consumed by the first it0 pops
                    for tt in range(2):
                        v65_cur[1][tt] = make_vt(1, tt)
                        for nk in range(2):
                            queue.append(
                                (lambda x=x_cur, vt=v65_cur[1][tt], t=tt, n=nk:
                                 emit_v_nk(x, vt, 1, t, n))
                            )

                if nxt is not None:
                    x_nxt = xp.tile([P, 8, W], BF16, tag="x")
                    nc.sync.dma_start(x_nxt, xT[:, nxt])
                    roped_nxt = rp.tile([P, 16, W], BF16)
                    for ft in range(16):
                        queue.append(
                            (lambda x=x_nxt, r=roped_nxt, f=ft:
                             emit_qk_ft(x, r, f))
                        )
                else:
                    # final phase it0 filler: fused proj of the previous pair
                    # as WHOLE chains, one per scores->AV slot (a 1.4us chain
                    # covers the exp latency; half-chains spread over both
                    # pop points undershoot it)
                    yF = yp.tile([P, 8, W], F32, tag="yF", bufs=1)
                    for ft in range(8):
                        queue.append(
                            (lambda c=concat_prev, y=yF, f=ft:
                             emit_proj_fused(c, y, f))
                        )
                    yF_prev = yF

                # it0 waves; in the last phase only the scores->AV pop is used
                for hp in range(8):
                    e0, e1 = emit_wave_sc(roped_cur, 0, hp)
                    if queue:
                        queue.popleft()()
                    emit_wave_av(v65_cur, concat, e0, e1, 0, hp)
                    if queue and nxt is not None:
                        queue.popleft()()

                if nxt is not None:
                    v65_nxt = [[None, None], [None, None]]
                    enq_v_units(x_nxt, v65_nxt, split=(pr == 0))
                    if pr >= 1:
                        # fused proj of pair pr-1 (whole chains)
                        yF = yp.tile([P, 8, W], F32, tag="yF", bufs=1)
                        for ft in range(8):
                            queue.append(
                                (lambda c=concat_prev, y=yF, f=ft:
                                 emit_proj_fused(c, y, f))
                            )
                        yF_prev = yF
                else:
                    v65_nxt = None
                    # deferred pair-2 output is fully emitted by now
                    nc.sync.dma_start(
                        out[:, NPAIR - 2],
                        yF_prev.rearrange("p f (it s) -> p f it s", it=2),
                    )
                    # final phase it1 filler: this pair's it0 proj as whole
                    # chains, one per scores->AV slot
                    y3 = yp.tile([P, 8, 2, S], F32, tag="y3", bufs=1)
                    for ft in range(8):
                        queue.append(
                            (lambda c=concat, y=y3, f=ft:
                             emit_proj(c, y, 0, f))
                        )

                # it1 waves
                for hp in range(8):
                    e0, e1 = emit_wave_sc(roped_cur, 1, hp)
                    if queue:
                        queue.popleft()()
                    emit_wave_av(v65_cur, concat, e0, e1, 1, hp)
                    if queue and nxt is not None:
                        queue.popleft()()

                # drain leftovers (pure chain work, gap-free)
                while queue:
                    queue.popleft()()

                if debug_taps and pr == 0:
                    nc.sync.dma_start(
                        dbg_roped.rearrange("p (f w) -> p f w", f=16), roped_cur
                    )
                    nc.sync.dma_start(
                        dbg_v.rearrange("p (h d) -> p h d", h=16), v65_cur[0][0]
                    )
                    nc.sync.dma_start(
                        dbg_concat.rearrange("p (j w) -> p j w", j=8), concat
                    )
                    nc.sync.dma_start(dbg_e[:, :], e0[:, 0])

                if nxt is not None:
                    if pr >= 1:
                        # pair pr-1's fused output is complete after the drain
                        nc.sync.dma_start(
                            out[:, pr - 1],
                            yF_prev.rearrange("p f (it s) -> p f it s", it=2),
                        )
                    concat_prev = concat
                    x_cur, roped_cur, v65_cur = x_nxt, roped_nxt, v65_nxt
                else:
                    # tail: this pair's it0 output, then bare it1 proj chains
                    # with the output streamed out in 2-ft chunks so the last
                    # DMA only moves ~200KB after the final IDENT
                    nc.sync.dma_start(out[:, pr, :, 0, :], y3[:, :, 0])
                    for ft in range(8):
                        emit_proj(concat, y3, 1, ft)
                        if ft % 2 == 1:
                            nc.sync.dma_start(
                                out[:, pr, ft - 1 : ft + 1, 1, :],
                                y3[:, ft - 1 : ft + 1, 1],
                            )

    nc.compile()
    return nc


def host_pack_inputs(x, qkv_w, qkv_b, proj_w, proj_b, n_items=NI):
    """Build per-core in_maps (host-side layout only, no math on x)."""
    perm, cosT, sinST, pmatT = _host_tables()
    wqk_dev, wv_dev, wpr_dev, bqk_dev, beff_dev = _pack_weights(
        qkv_w, qkv_b, proj_w, proj_b, perm
    )
    shared = {
        "wqk": np.ascontiguousarray(wqk_dev.reshape(P, -1)),
        "wv": np.ascontiguousarray(wv_dev.reshape(P, -1)),
        "wpr": np.ascontiguousarray(wpr_dev.reshape(P, -1)),
        "pmat": np.ascontiguousarray(pmatT),
        "bqk": bqk_dev,
        "beff": beff_dev,
        "cosT": np.ascontiguousarray(cosT),
        "sinST": np.ascontiguousarray(sinST),
    }
    n_cores = x.shape[0] // n_items
    in_maps = []
    for c in range(n_cores):
        xs = x[c * n_items : (c + 1) * n_items]  # [ni, 197, 1024]
        # device layout [p, pair, kt, w]: feature kt*128+p, token pair*394+w
        xt = xs.reshape(NPAIR, W, C).astype(BF16_NP)
        xt = xt.transpose(2, 0, 1).reshape(8, 128, NPAIR, W).transpose(1, 2, 0, 3)
        in_maps.append(
            {"xT": np.ascontiguousarray(xt.reshape(P, -1)), **shared}
        )
    return in_maps


def unpack_output(results, n_items=NI):
    """results: per-core {'out': [128, NPAIR, 2, 8, S]} -> full (B, N, C)."""
    outs = []
    for r in results:
        yT = r["out"].reshape(P, NPAIR, 8, 2, S)
        # feature o*128+p, token pr*394+it*197+s; device layout [p,pr,o,it,s]
        y = yT.transpose(1, 3, 4, 2, 0).reshape(n_items * S, C)
        outs.append(y.reshape(n_items, S, C))
    return np.concatenate(outs, axis=0)


_CACHED = {}


def kernel(x, qkv_w, qkv_b, proj_w, proj_b):
    from concourse.bass_utils import run_bass_kernel_spmd

    x = np.asarray(x, dtype=np.float32)
    qkv_w = np.asarray(qkv_w, dtype=np.float32)
    qkv_b = np.asarray(qkv_b, dtype=np.float32)
    proj_w = np.asarray(proj_w, dtype=np.float32)
    proj_b = np.asarray(proj_b, dtype=np.float32)

    if "nc" not in _CACHED:
        _CACHED["nc"] = build_nc(NI)
    nc = _CACHED["nc"]
    in_maps = host_pack_inputs(x, qkv_w, qkv_b, proj_w, proj_b, NI)
    res = run_bass_kernel_spmd(nc, in_maps, core_ids=list(range(N_CORES)))
    return unpack_output(res.results, NI).astype(np.float32)


if __name__ == "__main__":
    pass


# revision 55
# speedup vs baseline: 1.1966x; 1.0040x over previous
"""Trainium2 Bass kernel for ViT-style attention block with RoPE.

Problem: x(64,197,1024), qkv(3072x1024)+b, proj(1024x1024)+b, H=16 heads,
RoPE (interleaved pairs, tiled cos/sin tables) on all tokens but CLS.

Strategy: data-parallel over batch across 8 cores (8 items each, no
collectives). Host pre-transposes all operands so the device only runs
matmuls / softmax / RoPE in "transposed" layouts:

  - qk part:  qkT[f, t] = Wqk^T stationary x xT moving   (features on partitions)
  - v part:   v[t, f]   = xT stationary x Wv moving      (tokens on partitions)
  - scores:   scT[j, i] = kT(lhsT) x qT(rhs); per head one PSUM bank holds
              jt0 (cols 0:S) and jt1 (cols 256:256+S) as two non-accumulating
              full-overwrite matmuls (start=True resets has_written bits for
              the bank but not the data, so disjoint overwrites are safe;
              never pack two ACCUMULATING groups in one bank)
  - softmax:  exp on ScalarE (scale=1/8, no max subtraction; |logits|<~5),
              denominators via ones-columns PREPENDED to v (cols 0:64, so the
              sums land at partition base 0: reciprocal_approx_fast silently
              mis-reads partition-offset inputs on HW), normalization =
              reciprocal + DVE mult (standard DVE ops handle offsets fine)
  - AV:       per head a standard 2-matmul accumulation group in its own bank
  - RoPE:     q' = (q+b)*cos + (P(q+b))*sinS where P is a 128x128 block-swap
              permutation done on the TensorEngine; sign and d-permutation
              folded into host-built tables; elementwise split GpSimd/DVE
  - v bias:   folded into proj bias on host (attn rows sum to 1)
  - proj:     yT = Wproj^T stationary x concatT moving, bias on ScalarE;
              pairs 0..2 project both items in one N=394 chain (each weight
              tile loaded once -> half the LDWEIGHTS), shifted one phase later

Scheduling: a software-pipelined unit queue interleaves QKV/V/proj chain
work between each wave's scores and AV matmuls so the PE never waits on
the ScalarE exp; PSUM is exactly 8 banks: chains 2x[P,512] (bufs=3 incl.
spare), rope 1, scores [P,1024], AV [P,1024]. DRAM layouts make every DMA
contiguous per partition row (fewer descriptors; each dma_start costs
~0.6us of serial DIRECT2D descriptor generation on SyncE, so the startup
stream is ordered x0/fg0 first, then consts, then bulk weights).
"""

import sys
from collections import deque

for _p in ("/opt/trn_rl_repo", "/opt/pypackages"):
    if _p not in sys.path:
        sys.path.append(_p)

import numpy as np
import ml_dtypes

import concourse.bass as bass
import concourse.tile as tile
from concourse import bacc
from concourse import mybir

F32 = mybir.dt.float32
BF16 = mybir.dt.bfloat16
BF16_NP = ml_dtypes.bfloat16

# Problem constants (hardcoded per the contract)
B, N, C = 64, 197, 1024
H, D = 16, 64
E = 1  # CLS tokens
THETA = 10000.0
N_CORES = 8
NI = B // N_CORES  # items per core = 8
NT = NI * N  # tokens per core = 1576
S = N  # 197
W = 2 * S  # pair width = 394
NPAIR = NI // 2  # 4
P = 128


def _host_tables():
    """RoPE cos/sin in device layout + permutations, all position-only."""
    seq = (224 // 16) ** 2  # 196
    exp = np.arange(0, D, 2, dtype=np.float64) / -D
    base = THETA**exp  # (32,)
    t = np.arange(seq, dtype=np.float64)
    f0 = np.outer(t, base)  # (196, 32)
    f = np.concatenate([f0, f0], axis=-1)  # (196, 64) "tiled"
    cos_ref = np.cos(f)
    sin_ref = np.sin(f)

    # permutation: new dd<32 -> orig 2dd (x0), new dd>=32 -> orig 2(dd-32)+1 (x1)
    perm = np.empty(D, dtype=np.int64)
    perm[:32] = np.arange(32) * 2
    perm[32:] = np.arange(32) * 2 + 1

    # per-token columns for an item: col 0 = CLS (cos=1, sin=0), cols 1..196 = rope
    cos_item = np.ones((D, S), dtype=np.float64)
    sin_item = np.zeros((D, S), dtype=np.float64)
    cos_item[:, 1:] = cos_ref[:, perm].T
    sin_item[:, 1:] = sin_ref[:, perm].T
    # fold rotate-half signs into sin: rot[dd<32] = -q[dd+32], rot[dd>=32] = +q[dd-32]
    sinS_item = sin_item.copy()
    sinS_item[:32, :] *= -1.0

    # pair-width, replicated for the 2 heads in a 128-partition tile
    cosT = np.tile(cos_item, (2, 2)).astype(BF16_NP)  # [128, 394]
    sinST = np.tile(sinS_item, (2, 2)).astype(BF16_NP)  # [128, 394]

    # 128x128 swap permutation (block swap +-32 within each 64-head-half),
    # already transposed for use as lhsT: rot = P @ q  ->  lhsT = P.T
    Pm = np.zeros((P, P), dtype=np.float32)
    for p in range(P):
        src = 64 * (p // 64) + ((p % 64) + 32) % 64
        Pm[p, src] = 1.0
    pmatT = Pm.T.astype(BF16_NP)  # [K=128, M=128]

    return perm, cosT, sinST, pmatT


def _pack_weights(qkv_w, qkv_b, proj_w, proj_b, perm):
    """Host-side weight packing into device layouts (all numpy, one-time)."""
    # feature permutation for q/k heads: rows of qkv_w within each head
    qk_perm = np.concatenate(
        [h * D + perm for h in range(2 * H)]  # q heads then k heads
    )
    wqk = qkv_w[:2048][qk_perm]  # (2048, 1024) permuted
    bqk = qkv_b[:2048][qk_perm]  # (2048,)
    wv = qkv_w[2048:]  # (1024, 1024)
    bv = qkv_b[2048:]

    wqk_T = np.ascontiguousarray(wqk.T).astype(BF16_NP)  # [1024 k, 2048 f]
    wv_T = np.ascontiguousarray(wv.T).astype(BF16_NP)  # [1024 k, 1024 f]
    proj_wT = np.ascontiguousarray(proj_w.T).astype(BF16_NP)  # [1024, 1024]

    # device layouts with contiguous per-partition DMA blocks:
    # wqk: [p, fg, o, 256] with k = o*128+p, f = fg*256+c
    wqk_dev = np.ascontiguousarray(
        wqk_T.reshape(8, 128, 8, 256).transpose(1, 2, 0, 3)
    )
    # wv/wpr: [p, o, 1024]
    wv_dev = np.ascontiguousarray(wv_T.reshape(8, 128, 1024).transpose(1, 0, 2))
    wpr_dev = np.ascontiguousarray(proj_wT.reshape(8, 128, 1024).transpose(1, 0, 2))

    # biases in [128, ftile] per-partition layout
    bqk_dev = np.ascontiguousarray(bqk.reshape(16, 128).T).astype(np.float32)
    # v bias folded into proj bias: y = concat@W^T + (W@bv + pb)
    beff = proj_w.astype(np.float64) @ bv.astype(np.float64) + proj_b
    beff_dev = np.ascontiguousarray(beff.reshape(8, 128).T).astype(np.float32)
    return wqk_dev, wv_dev, wpr_dev, bqk_dev, beff_dev


def build_nc(n_items=NI, debug_taps=False):
    """Build the per-core Bass graph. SPMD: same graph on all cores."""
    assert n_items == NI
    nc = bacc.Bacc(None, target_bir_lowering=False, debug=False)

    # all DRAM params are 2-D [P, cols]; >2-D shapes are realized as device-
    # side rearranges (the HW ingestion path lays out 2-D exactly like the
    # host's C-order arrays)
    xT2 = nc.declare_dram_parameter("xT", [P, NPAIR * 8 * W], BF16, isOutput=False)
    wqk2 = nc.declare_dram_parameter("wqk", [P, 8 * 8 * 256], BF16, isOutput=False)
    wv2 = nc.declare_dram_parameter("wv", [P, 8 * C], BF16, isOutput=False)
    wpr2 = nc.declare_dram_parameter("wpr", [P, 8 * C], BF16, isOutput=False)
    pmat = nc.declare_dram_parameter("pmat", [P, P], BF16, isOutput=False)
    bqk = nc.declare_dram_parameter("bqk", [P, 16], F32, isOutput=False)
    beff = nc.declare_dram_parameter("beff", [P, 8], F32, isOutput=False)
    cosT = nc.declare_dram_parameter("cosT", [P, W], BF16, isOutput=False)
    sinST = nc.declare_dram_parameter("sinST", [P, W], BF16, isOutput=False)
    out2 = nc.declare_dram_parameter(
        "out", [P, NPAIR * 2 * 8 * S], F32, isOutput=True
    )
    if debug_taps:
        dbg_roped = nc.declare_dram_parameter("dbg_roped", [P, 16 * W], BF16, True)
        dbg_v = nc.declare_dram_parameter("dbg_v", [P, 16 * 128], BF16, True)
        dbg_concat = nc.declare_dram_parameter("dbg_concat", [P, 8 * W], BF16, True)
        dbg_e = nc.declare_dram_parameter("dbg_e", [P, S], BF16, True)
    xT = xT2.rearrange("p (pr k w) -> p pr k w", pr=NPAIR, k=8)
    wqk = wqk2.rearrange("p (fg o c) -> p fg o c", fg=8, o=8)
    wv = wv2.rearrange("p (o f) -> p o f", o=8)
    wpr = wpr2.rearrange("p (o f) -> p o f", o=8)
    out = out2.rearrange("p (pr o it s) -> p pr o it s", pr=NPAIR, o=8, it=2)

    Exp = mybir.ActivationFunctionType.Exp
    Ident = mybir.ActivationFunctionType.Identity

    with tile.TileContext(nc) as tc:
        with (
            tc.tile_pool(name="const", bufs=1) as const,
            tc.tile_pool(name="xp", bufs=2) as xp,
            tc.tile_pool(name="roped", bufs=2) as rp,
            tc.tile_pool(name="vp", bufs=2) as vp,
            tc.tile_pool(name="work", bufs=2) as wk,
            tc.tile_pool(name="ep", bufs=2) as ep,
            tc.tile_pool(name="cc", bufs=2) as cc,
            tc.tile_pool(name="yp", bufs=2) as yp,
            tc.tile_pool(name="psC", bufs=3, space="PSUM") as psC,
            tc.tile_pool(name="psR", bufs=1, space="PSUM") as psR,
            tc.tile_pool(name="psS", bufs=1, space="PSUM") as psS,
            tc.tile_pool(name="psV", bufs=1, space="PSUM") as psV,
        ):
            # ---- early DMAs: the first chain's inputs (x0, fg0) lead, then
            # the consts (needed ~2us later by the first IDENT/rope), then
            # the rest of the wqk stream and v/proj weights ----
            x_first = xp.tile([P, 8, W], BF16, tag="x")
            wqk_sb = const.tile([P, 8, 8, 256], BF16)
            nc.sync.dma_start(x_first, xT[:, 0])
            nc.sync.dma_start(wqk_sb[:, 0], wqk[:, 0])

            bqk_sb = const.tile([P, 16], F32)
            nc.sync.dma_start(bqk_sb, bqk[:, :])
            beff_sb = const.tile([P, 8], F32)
            nc.sync.dma_start(beff_sb, beff[:, :])
            cos_sb = const.tile([P, W], BF16)
            nc.sync.dma_start(cos_sb, cosT[:, :])
            sin_sb = const.tile([P, W], BF16)
            nc.sync.dma_start(sin_sb, sinST[:, :])
            pmat_sb = const.tile([P, P], BF16)
            nc.sync.dma_start(pmat_sb, pmat[:, :])

            # warm the activation table so the 1.3us table load is off the
            # critical path of the first bias-add
            dummy = const.tile([P, 8], F32)
            nc.gpsimd.memset(dummy, 0.0)
            dummy2 = const.tile([P, 8], F32)
            nc.scalar.activation(dummy2, dummy, Exp, scale=0.125)

            for fg in range(1, 8):
                nc.sync.dma_start(wqk_sb[:, fg], wqk[:, fg])
            wv_sb = const.tile([P, 8, C], BF16)
            nc.sync.dma_start(wv_sb, wv[:, :, :])
            wpr_sb = const.tile([P, 8, C], BF16)
            nc.sync.dma_start(wpr_sb, wpr[:, :, :])

            # ---- emission helpers ----
            def emit_qk_ft(x_sb, roped, ft):
                ps = psC.tile([P, 512], F32, tag="chain")
                fg, half = ft // 2, ft % 2
                for kt in range(8):
                    nc.tensor.matmul(
                        ps[:, 0:W],
                        wqk_sb[:, fg, kt, half * 128 : (half + 1) * 128],
                        x_sb[:, kt, :],
                        start=(kt == 0),
                        stop=(kt == 7),
                    )
                tmp = wk.tile([P, W], BF16, tag="tmp")
                nc.scalar.activation(
                    tmp, ps[:, 0:W], Ident, bias=bqk_sb[:, ft : ft + 1]
                )
                psr = psR.tile([P, 512], F32, tag="rope")
                nc.tensor.matmul(psr[:, 0:W], pmat_sb, tmp, start=True, stop=True)
                acc = wk.tile([P, W], BF16, tag="acc")
                nc.gpsimd.tensor_mul(acc, tmp, cos_sb)
                rot2 = wk.tile([P, W], BF16, tag="rot2")
                nc.vector.tensor_mul(rot2, psr[:, 0:W], sin_sb)
                nc.gpsimd.tensor_add(roped[:, ft, :], acc, rot2)

            def make_vt(it2, tt):
                pcount = 128 if tt == 0 else 69
                vt = vp.tile([P, 16, 128], BF16, tag=f"v{it2}{tt}")
                return vt

            def emit_v_nk(x_sb, vt, it2, tt, nk, kts=range(8), ps=None):
                pcount = 128 if tt == 0 else 69
                kts = list(kts)
                if ps is None:
                    ps = psC.tile([P, 512], F32, tag="chain")
                for kt in kts:
                    nc.tensor.matmul(
                        ps[:pcount, :],
                        x_sb[:, kt, it2 * S + tt * P : it2 * S + tt * P + pcount],
                        wv_sb[:, kt, nk * 512 : (nk + 1) * 512],
                        start=(kt == 0),
                        stop=(kt == 7),
                    )
                if kts[-1] != 7:
                    return ps
                # ones in cols 0:64, features in 64:128 so the AV output has
                # the softmax sums at partition base 0 (reciprocal_approx_fast
                # mis-reads partition-offset inputs on HW)
                nc.vector.tensor_copy(
                    vt[:pcount, nk * 8 : (nk + 1) * 8, 64:128],
                    ps[:pcount, :].rearrange("p (h d) -> p h d", d=64),
                )
                if nk == 1:
                    nc.gpsimd.memset(vt[:pcount, :, 0:64], 1.0)

            def emit_proj_fused(concat, y, ft, kts=range(8), ps=None):
                """proj for BOTH items of a pair at once: N=W columns, so
                each weight tile is loaded once for 394 output columns
                (halves LDWEIGHTS traffic vs per-item chains)."""
                kts = list(kts)
                if ps is None:
                    ps = psC.tile([P, 512], F32, tag="chain")
                for kt in kts:
                    nc.tensor.matmul(
                        ps[:, 0:W],
                        wpr_sb[:, kt, ft * 128 : (ft + 1) * 128],
                        concat[:, kt, :],
                        start=(kt == 0),
                        stop=(kt == 7),
                    )
                if kts[-1] == 7:
                    nc.scalar.activation(
                        y[:, ft, :], ps[:, 0:W], Ident, bias=beff_sb[:, ft : ft + 1]
                    )
                return ps

            def emit_proj(concat, y, it2, ft, kts=range(8), ps=None):
                ts = it2 * S
                kts = list(kts)
                if ps is None:
                    ps = psC.tile([P, 512], F32, tag="chain")
                for kt in kts:
                    nc.tensor.matmul(
                        ps[:, 0:S],
                        wpr_sb[:, kt, ft * 128 : (ft + 1) * 128],
                        concat[:, kt, ts : ts + S],
                        start=(kt == 0),
                        stop=(kt == 7),
                    )
                if kts[-1] == 7:
                    nc.scalar.activation(
                        y[:, ft, it2, :], ps[:, 0:S], Ident,
                        bias=beff_sb[:, ft : ft + 1],
                    )
                return ps

            def emit_wave_sc(roped, it2, hp):
                """Scores for head pair hp. Per head one PSUM bank holding
                jt0 at cols 0:S and jt1 at cols 256:256+S. Both writes are
                full overwrites of disjoint ranges (start=True stop=True),
                never accumulation, so sharing the bank is safe on HW (start
                clears has_written bits, not data)."""
                ts = it2 * S
                sc = psS.tile([P, 1024], F32, tag="sc")
                kqs = []
                for bk, h in ((0, 2 * hp), (1, 2 * hp + 1)):
                    hb = 64 * (h % 2)
                    kT = roped[hb : hb + 64, 8 + h // 2, ts : ts + S]
                    qT = roped[hb : hb + 64, h // 2, ts : ts + S]
                    kqs.append((bk, kT, qT))
                # jt0 of both heads first so e0 is ready one matmul earlier
                for bk, kT, qT in kqs:
                    nc.tensor.matmul(
                        sc[:, bk * 512 : bk * 512 + S], kT[:, 0:P], qT,
                        start=True, stop=True, skip_group_check=True,
                    )
                for bk, kT, qT in kqs:
                    nc.tensor.matmul(
                        sc[0:69, bk * 512 + 256 : bk * 512 + 256 + S],
                        kT[:, P:S], qT,
                        start=True, stop=True, skip_group_check=True,
                    )
                sc4 = sc.rearrange("p (b c) -> p b c", b=2)
                # ONE exp spanning jt0+jt1 (cols 0:453) of both banks: AV's
                # jt1 matmuls then wait one EXP (~830ns) instead of two
                # serial ones (~1170ns). The pad cols 197:256 and rows 69:128
                # of the jt1 range hold stale psum -> exp garbage in unused
                # e regions, harmless on HW.
                e = ep.tile([P, 2, 256 + S], BF16, tag="e")
                nc.scalar.activation(e, sc4[:, :, 0 : 256 + S], Exp, scale=0.125)
                return e[:, :, 0:S], e[:, :, 256 : 256 + S]

            def emit_wave_av(v65, concat, e0, e1, it2, hp):
                """AV + normalization for a head pair; per head one standard
                2-matmul accumulation group in its own bank."""
                ts = it2 * S
                hA, hB = 2 * hp, 2 * hp + 1
                av = psV.tile([P, 1024], F32, tag="av")
                nc.tensor.matmul(
                    av[:, 0:S], v65[it2][0][:, hA, :], e0[:, 0],
                    start=True, stop=False,
                )
                nc.tensor.matmul(
                    av[:, 512 : 512 + S], v65[it2][0][:, hB, :], e0[:, 1],
                    start=True, stop=False,
                )
                nc.tensor.matmul(
                    av[:, 0:S], v65[it2][1][0:69, hA, :], e1[0:69, 0],
                    start=False, stop=True,
                )
                nc.tensor.matmul(
                    av[:, 512 : 512 + S], v65[it2][1][0:69, hB, :], e1[0:69, 1],
                    start=False, stop=True,
                )
                av4 = av.rearrange("p (b c) -> p b c", b=2)
                rb = wk.tile([64, 2, S], F32, tag="rb", bufs=1)
                nc.vector.reciprocal_approx_fast(rb, av4[0:64, :, 0:S])
                nc.vector.tensor_mul(
                    concat[0:64, hp, ts : ts + S], av4[64:128, 0, 0:S], rb[:, 0]
                )
                nc.vector.tensor_mul(
                    concat[64:128, hp, ts : ts + S], av4[64:128, 1, 0:S], rb[:, 1]
                )

            # ---- prologue: pair-0 QKV + V(it0), bare (DMA-paced anyway) ----
            roped_cur = rp.tile([P, 16, W], BF16)
            for ft in range(16):
                emit_qk_ft(x_first, roped_cur, ft)
            v65_cur = [[None, None], [None, None]]
            for tt in range(2):
                v65_cur[0][tt] = make_vt(0, tt)
                for nk in range(2):
                    emit_v_nk(x_first, v65_cur[0][tt], 0, tt, nk)

            # ---- software-pipelined pair phases with a filler-unit queue ----
            queue = deque()
            x_cur = x_first

            def enq_split(fn):
                """Enqueue a chain as two 4-matmul halves sharing one psum
                tile (finer filler granularity for slot-starved phases)."""
                sh = {}

                def a():
                    sh["ps"] = fn(kts=range(0, 4))

                def b():
                    fn(kts=range(4, 8), ps=sh["ps"])

                queue.append(a)
                queue.append(b)

            def enq_v_units(x_sb, v65dst, split):
                for it2 in range(2):
                    for tt in range(2):
                        v65dst[it2][tt] = make_vt(it2, tt)
                        for nk in range(2):
                            fn = (
                                lambda x=x_sb, vt=v65dst[it2][tt], i=it2,
                                t=tt, n=nk, **kw: emit_v_nk(x, vt, i, t, n, **kw)
                            )
                            if split:
                                enq_split(fn)
                            else:
                                queue.append(fn)

            concat_prev = None
            yF_prev = None

            for pr in range(NPAIR):
                nxt = pr + 1 if pr + 1 < NPAIR else None
                concat = cc.tile([P, 8, W], BF16)

                if pr == 0:
                    # v65 for pair-0 it1, consumed by the first it0 pops
                    for tt in range(2):
                        v65_cur[1][tt] = make_vt(1, tt)
                        for nk in range(2):
                            queue.append(
                                (lambda x=x_cur, vt=v65_cur[1][tt], t=tt, n=nk:
                                 emit_v_nk(x, vt, 1, t, n))
                            )

                if nxt is not None:
                    x_nxt = xp.tile([P, 8, W], BF16, tag="x")
                    nc.sync.dma_start(x_nxt, xT[:, nxt])
                    roped_nxt = rp.tile([P, 16, W], BF16)
                    for ft in range(16):
                        queue.append(
                            (lambda x=x_nxt, r=roped_nxt, f=ft:
                             emit_qk_ft(x, r, f))
                        )
                else:
                    # final phase it0 filler: fused proj of the previous pair
                    # as WHOLE chains, one per scores->AV slot (a 1.4us chain
                    # covers the exp latency; half-chains spread over both
                    # pop points undershoot it)
                    yF = yp.tile([P, 8, W], F32, tag="yF", bufs=1)
                    for ft in range(8):
                        queue.append(
                            (lambda c=concat_prev, y=yF, f=ft:
                             emit_proj_fused(c, y, f))
                        )
                    yF_prev = yF

                # it0 waves; in the last phase only the scores->AV pop is used
                for hp in range(8):
                    e0, e1 = emit_wave_sc(roped_cur, 0, hp)
                    if queue:
                        queue.popleft()()
                    emit_wave_av(v65_cur, concat, e0, e1, 0, hp)
                    if queue and nxt is not None:
                        queue.popleft()()

                if nxt is not None:
                    v65_nxt = [[None, None], [None, None]]
                    enq_v_units(x_nxt, v65_nxt, split=(pr == 0))
                    if pr >= 1:
                        # fused proj of pair pr-1 (whole chains)
                        yF = yp.tile([P, 8, W], F32, tag="yF", bufs=1)
                        for ft in range(8):
                            queue.append(
                                (lambda c=concat_prev, y=yF, f=ft:
                                 emit_proj_fused(c, y, f))
                            )
                        yF_prev = yF
                else:
                    v65_nxt = None
                    # deferred pair-2 output is fully emitted by now
                    nc.sync.dma_start(
                        out[:, NPAIR - 2],
                        yF_prev.rearrange("p f (it s) -> p f it s", it=2),
                    )
                    # final phase it1 filler: this pair's it0 proj as whole
                    # chains, one per scores->AV slot
                    y3 = yp.tile([P, 8, 2, S], F32, tag="y3", bufs=1)
                    for ft in range(8):
                        queue.append(
                            (lambda c=concat, y=y3, f=ft:
                             emit_proj(c, y, 0, f))
                        )

                # it1 waves
                for hp in range(8):
                    e0, e1 = emit_wave_sc(roped_cur, 1, hp)
                    if queue:
                        queue.popleft()()
                    emit_wave_av(v65_cur, concat, e0, e1, 1, hp)
                    if queue and nxt is not None:
                        queue.popleft()()

                # drain leftovers (pure chain work, gap-free)
                while queue:
                    queue.popleft()()

                if debug_taps and pr == 0:
                    nc.sync.dma_start(
                        dbg_roped.rearrange("p (f w) -> p f w", f=16), roped_cur
                    )
                    nc.sync.dma_start(
                        dbg_v.rearrange("p (h d) -> p h d", h=16), v65_cur[0][0]
                    )
                    nc.sync.dma_start(
                        dbg_concat.rearrange("p (j w) -> p j w", j=8), concat
                    )
                    nc.sync.dma_start(dbg_e[:, :], e0[:, 0])

                if nxt is not None:
                    if pr >= 1:
                        # pair pr-1's fused output is complete after the drain
                        nc.sync.dma_start(
                            out[:, pr - 1],
                            yF_prev.rearrange("p f (it s) -> p f it s", it=2),
                        )
                    concat_prev = concat
                    x_cur, roped_cur, v65_cur = x_nxt, roped_nxt, v65_nxt
                else:
                    # tail: this pair's it0 output, then bare it1 proj chains
                    # with the output streamed out in 2-ft chunks so the last
                    # DMA only moves ~200KB after the final IDENT
                    nc.sync.dma_start(out[:, pr, :, 0, :], y3[:, :, 0])
                    for ft in range(8):
                        emit_proj(concat, y3, 1, ft)
                        if ft % 2 == 1:
                            nc.sync.dma_start(
                                out[:, pr, ft - 1 : ft + 1, 1, :],
                                y3[:, ft - 1 : ft + 1, 1],
                            )

    nc.compile()
    return nc


def host_pack_inputs(x, qkv_w, qkv_b, proj_w, proj_b, n_items=NI):
    """Build per-core in_maps (host-side layout only, no math on x)."""
    perm, cosT, sinST, pmatT = _host_tables()
    wqk_dev, wv_dev, wpr_dev, bqk_dev, beff_dev = _pack_weights(
        qkv_w, qkv_b, proj_w, proj_b, perm
    )
    shared = {
        "wqk": np.ascontiguousarray(wqk_dev.reshape(P, -1)),
        "wv": np.ascontiguousarray(wv_dev.reshape(P, -1)),
        "wpr": np.ascontiguousarray(wpr_dev.reshape(P, -1)),
        "pmat": np.ascontiguousarray(pmatT),
        "bqk": bqk_dev,
        "beff": beff_dev,
        "cosT": np.ascontiguousarray(cosT),
        "sinST": np.ascontiguousarray(sinST),
    }
    n_cores = x.shape[0] // n_items
    in_maps = []
    for c in range(n_cores):
        xs = x[c * n_items : (c + 1) * n_items]  # [ni, 197, 1024]
        # device layout [p, pair, kt, w]: feature kt*128+p, token pair*394+w
        xt = xs.reshape(NPAIR, W, C).astype(BF16_NP)
        xt = xt.transpose(2, 0, 1).reshape(8, 128, NPAIR, W).transpose(1, 2, 0, 3)
        in_maps.append(
            {"xT": np.ascontiguousarray(xt.reshape(P, -1)), **shared}
        )
    return in_maps


def unpack_output(results, n_items=NI):
    """results: per-core {'out': [128, NPAIR, 2, 8, S]} -> full (B, N, C)."""
    outs = []
    for r in results:
        yT = r["out"].reshape(P, NPAIR, 8, 2, S)
        # feature o*128+p, token pr*394+it*197+s; device layout [p,pr,o,it,s]
        y = yT.transpose(1, 3, 4, 2, 0).reshape(n_items * S, C)
        outs.append(y.reshape(n_items, S, C))
    return np.concatenate(outs, axis=0)


_CACHED = {}


def kernel(x, qkv_w, qkv_b, proj_w, proj_b):
    from concourse.bass_utils import run_bass_kernel_spmd

    x = np.asarray(x, dtype=np.float32)
    qkv_w = np.asarray(qkv_w, dtype=np.float32)
    qkv_b = np.asarray(qkv_b, dtype=np.float32)
    proj_w = np.asarray(proj_w, dtype=np.float32)
    proj_b = np.asarray(proj_b, dtype=np.float32)

    if "nc" not in _CACHED:
        _CACHED["nc"] = build_nc(NI)
    nc = _CACHED["nc"]
    in_maps = host_pack_inputs(x, qkv_w, qkv_b, proj_w, proj_b, NI)
    res = run_bass_kernel_spmd(nc, in_maps, core_ids=list(range(N_CORES)))
    return unpack_output(res.results, NI).astype(np.float32)


if __name__ == "__main__":
    pass


# revision 56
# speedup vs baseline: 1.1978x; 1.0010x over previous
"""Trainium2 Bass kernel for ViT-style attention block with RoPE.

Problem: x(64,197,1024), qkv(3072x1024)+b, proj(1024x1024)+b, H=16 heads,
RoPE (interleaved pairs, tiled cos/sin tables) on all tokens but CLS.

Strategy: data-parallel over batch across 8 cores (8 items each, no
collectives). Host pre-transposes all operands so the device only runs
matmuls / softmax / RoPE in "transposed" layouts:

  - qk part:  qkT[f, t] = Wqk^T stationary x xT moving   (features on partitions)
  - v part:   v[t, f]   = xT stationary x Wv moving      (tokens on partitions)
  - scores:   scT[j, i] = kT(lhsT) x qT(rhs); per head one PSUM bank holds
              jt0 (cols 0:S) and jt1 (cols 256:256+S) as two non-accumulating
              full-overwrite matmuls (start=True resets has_written bits for
              the bank but not the data, so disjoint overwrites are safe;
              never pack two ACCUMULATING groups in one bank)
  - softmax:  exp on ScalarE (scale=1/8, no max subtraction; |logits|<~5),
              denominators via ones-columns PREPENDED to v (cols 0:64, so the
              sums land at partition base 0: reciprocal_approx_fast silently
              mis-reads partition-offset inputs on HW), normalization =
              reciprocal + DVE mult (standard DVE ops handle offsets fine)
  - AV:       per head a standard 2-matmul accumulation group in its own bank
  - RoPE:     q' = (q+b)*cos + (P(q+b))*sinS where P is a 128x128 block-swap
              permutation done on the TensorEngine; sign and d-permutation
              folded into host-built tables; elementwise split GpSimd/DVE
  - v bias:   folded into proj bias on host (attn rows sum to 1)
  - proj:     yT = Wproj^T stationary x concatT moving, bias on ScalarE;
              pairs 0..2 project both items in one N=394 chain (each weight
              tile loaded once -> half the LDWEIGHTS), shifted one phase later

Scheduling: a software-pipelined unit queue interleaves QKV/V/proj chain
work between each wave's scores and AV matmuls so the PE never waits on
the ScalarE exp; PSUM is exactly 8 banks: chains 2x[P,512] (bufs=3 incl.
spare), rope 1, scores [P,1024], AV [P,1024]. DRAM layouts make every DMA
contiguous per partition row (fewer descriptors; each dma_start costs
~0.6us of serial DIRECT2D descriptor generation on SyncE, so the startup
stream is ordered x0/fg0 first, then consts, then bulk weights).
"""

import sys
from collections import deque

for _p in ("/opt/trn_rl_repo", "/opt/pypackages"):
    if _p not in sys.path:
        sys.path.append(_p)

import numpy as np
import ml_dtypes

import concourse.bass as bass
import concourse.tile as tile
from concourse import bacc
from concourse import mybir

F32 = mybir.dt.float32
BF16 = mybir.dt.bfloat16
BF16_NP = ml_dtypes.bfloat16

# Problem constants (hardcoded per the contract)
B, N, C = 64, 197, 1024
H, D = 16, 64
E = 1  # CLS tokens
THETA = 10000.0
N_CORES = 8
NI = B // N_CORES  # items per core = 8
NT = NI * N  # tokens per core = 1576
S = N  # 197
W = 2 * S  # pair width = 394
NPAIR = NI // 2  # 4
P = 128


def _host_tables():
    """RoPE cos/sin in device layout + permutations, all position-only."""
    seq = (224 // 16) ** 2  # 196
    exp = np.arange(0, D, 2, dtype=np.float64) / -D
    base = THETA**exp  # (32,)
    t = np.arange(seq, dtype=np.float64)
    f0 = np.outer(t, base)  # (196, 32)
    f = np.concatenate([f0, f0], axis=-1)  # (196, 64) "tiled"
    cos_ref = np.cos(f)
    sin_ref = np.sin(f)

    # permutation: new dd<32 -> orig 2dd (x0), new dd>=32 -> orig 2(dd-32)+1 (x1)
    perm = np.empty(D, dtype=np.int64)
    perm[:32] = np.arange(32) * 2
    perm[32:] = np.arange(32) * 2 + 1

    # per-token columns for an item: col 0 = CLS (cos=1, sin=0), cols 1..196 = rope
    cos_item = np.ones((D, S), dtype=np.float64)
    sin_item = np.zeros((D, S), dtype=np.float64)
    cos_item[:, 1:] = cos_ref[:, perm].T
    sin_item[:, 1:] = sin_ref[:, perm].T
    # fold rotate-half signs into sin: rot[dd<32] = -q[dd+32], rot[dd>=32] = +q[dd-32]
    sinS_item = sin_item.copy()
    sinS_item[:32, :] *= -1.0

    # pair-width, replicated for the 2 heads in a 128-partition tile
    cosT = np.tile(cos_item, (2, 2)).astype(BF16_NP)  # [128, 394]
    sinST = np.tile(sinS_item, (2, 2)).astype(BF16_NP)  # [128, 394]

    # 128x128 swap permutation (block swap +-32 within each 64-head-half),
    # already transposed for use as lhsT: rot = P @ q  ->  lhsT = P.T
    Pm = np.zeros((P, P), dtype=np.float32)
    for p in range(P):
        src = 64 * (p // 64) + ((p % 64) + 32) % 64
        Pm[p, src] = 1.0
    pmatT = Pm.T.astype(BF16_NP)  # [K=128, M=128]

    return perm, cosT, sinST, pmatT


def _pack_weights(qkv_w, qkv_b, proj_w, proj_b, perm):
    """Host-side weight packing into device layouts (all numpy, one-time)."""
    # feature permutation for q/k heads: rows of qkv_w within each head
    qk_perm = np.concatenate(
        [h * D + perm for h in range(2 * H)]  # q heads then k heads
    )
    wqk = qkv_w[:2048][qk_perm]  # (2048, 1024) permuted
    bqk = qkv_b[:2048][qk_perm]  # (2048,)
    wv = qkv_w[2048:]  # (1024, 1024)
    bv = qkv_b[2048:]

    wqk_T = np.ascontiguousarray(wqk.T).astype(BF16_NP)  # [1024 k, 2048 f]
    wv_T = np.ascontiguousarray(wv.T).astype(BF16_NP)  # [1024 k, 1024 f]
    proj_wT = np.ascontiguousarray(proj_w.T).astype(BF16_NP)  # [1024, 1024]

    # device layouts with contiguous per-partition DMA blocks:
    # wqk: [p, fg, o, 256] with k = o*128+p, f = fg*256+c
    wqk_dev = np.ascontiguousarray(
        wqk_T.reshape(8, 128, 8, 256).transpose(1, 2, 0, 3)
    )
    # wv/wpr: [p, o, 1024]
    wv_dev = np.ascontiguousarray(wv_T.reshape(8, 128, 1024).transpose(1, 0, 2))
    wpr_dev = np.ascontiguousarray(proj_wT.reshape(8, 128, 1024).transpose(1, 0, 2))

    # biases in [128, ftile] per-partition layout
    bqk_dev = np.ascontiguousarray(bqk.reshape(16, 128).T).astype(np.float32)
    # v bias folded into proj bias: y = concat@W^T + (W@bv + pb)
    beff = proj_w.astype(np.float64) @ bv.astype(np.float64) + proj_b
    beff_dev = np.ascontiguousarray(beff.reshape(8, 128).T).astype(np.float32)
    return wqk_dev, wv_dev, wpr_dev, bqk_dev, beff_dev


def build_nc(n_items=NI, debug_taps=False):
    """Build the per-core Bass graph. SPMD: same graph on all cores."""
    assert n_items == NI
    nc = bacc.Bacc(None, target_bir_lowering=False, debug=False)

    # all DRAM params are 2-D [P, cols]; >2-D shapes are realized as device-
    # side rearranges (the HW ingestion path lays out 2-D exactly like the
    # host's C-order arrays)
    xT2 = nc.declare_dram_parameter("xT", [P, NPAIR * 8 * W], BF16, isOutput=False)
    wqk2 = nc.declare_dram_parameter("wqk", [P, 8 * 8 * 256], BF16, isOutput=False)
    wv2 = nc.declare_dram_parameter("wv", [P, 8 * C], BF16, isOutput=False)
    wpr2 = nc.declare_dram_parameter("wpr", [P, 8 * C], BF16, isOutput=False)
    pmat = nc.declare_dram_parameter("pmat", [P, P], BF16, isOutput=False)
    bqk = nc.declare_dram_parameter("bqk", [P, 16], F32, isOutput=False)
    beff = nc.declare_dram_parameter("beff", [P, 8], F32, isOutput=False)
    cosT = nc.declare_dram_parameter("cosT", [P, W], BF16, isOutput=False)
    sinST = nc.declare_dram_parameter("sinST", [P, W], BF16, isOutput=False)
    out2 = nc.declare_dram_parameter(
        "out", [P, NPAIR * 2 * 8 * S], F32, isOutput=True
    )
    if debug_taps:
        dbg_roped = nc.declare_dram_parameter("dbg_roped", [P, 16 * W], BF16, True)
        dbg_v = nc.declare_dram_parameter("dbg_v", [P, 16 * 128], BF16, True)
        dbg_concat = nc.declare_dram_parameter("dbg_concat", [P, 8 * W], BF16, True)
        dbg_e = nc.declare_dram_parameter("dbg_e", [P, S], BF16, True)
    xT = xT2.rearrange("p (pr k w) -> p pr k w", pr=NPAIR, k=8)
    wqk = wqk2.rearrange("p (fg o c) -> p fg o c", fg=8, o=8)
    wv = wv2.rearrange("p (o f) -> p o f", o=8)
    wpr = wpr2.rearrange("p (o f) -> p o f", o=8)
    out = out2.rearrange("p (pr o it s) -> p pr o it s", pr=NPAIR, o=8, it=2)

    Exp = mybir.ActivationFunctionType.Exp
    Ident = mybir.ActivationFunctionType.Identity

    with tile.TileContext(nc) as tc:
        with (
            tc.tile_pool(name="const", bufs=1) as const,
            tc.tile_pool(name="xp", bufs=2) as xp,
            tc.tile_pool(name="roped", bufs=2) as rp,
            tc.tile_pool(name="vp", bufs=2) as vp,
            tc.tile_pool(name="work", bufs=2) as wk,
            tc.tile_pool(name="ep", bufs=2) as ep,
            tc.tile_pool(name="cc", bufs=2) as cc,
            tc.tile_pool(name="yp", bufs=2) as yp,
            tc.tile_pool(name="psC", bufs=3, space="PSUM") as psC,
            tc.tile_pool(name="psR", bufs=1, space="PSUM") as psR,
            tc.tile_pool(name="psS", bufs=1, space="PSUM") as psS,
            tc.tile_pool(name="psV", bufs=1, space="PSUM") as psV,
        ):
            # ---- early DMAs: the first chain's inputs (x0, fg0) lead, then
            # the consts (needed ~2us later by the first IDENT/rope), then
            # the rest of the wqk stream and v/proj weights ----
            x_first = xp.tile([P, 8, W], BF16, tag="x")
            wqk_sb = const.tile([P, 8, 8, 256], BF16)
            nc.sync.dma_start(x_first, xT[:, 0])
            nc.sync.dma_start(wqk_sb[:, 0], wqk[:, 0])

            bqk_sb = const.tile([P, 16], F32)
            nc.sync.dma_start(bqk_sb, bqk[:, :])
            beff_sb = const.tile([P, 8], F32)
            nc.sync.dma_start(beff_sb, beff[:, :])
            cos_sb = const.tile([P, W], BF16)
            nc.sync.dma_start(cos_sb, cosT[:, :])
            sin_sb = const.tile([P, W], BF16)
            nc.sync.dma_start(sin_sb, sinST[:, :])
            pmat_sb = const.tile([P, P], BF16)
            nc.sync.dma_start(pmat_sb, pmat[:, :])

            # warm the activation table so the 1.3us table load is off the
            # critical path of the first bias-add
            dummy = const.tile([P, 8], F32)
            nc.gpsimd.memset(dummy, 0.0)
            dummy2 = const.tile([P, 8], F32)
            nc.scalar.activation(dummy2, dummy, Exp, scale=0.125)

            for fg in range(1, 8):
                nc.sync.dma_start(wqk_sb[:, fg], wqk[:, fg])
            wv_sb = const.tile([P, 8, C], BF16)
            nc.sync.dma_start(wv_sb, wv[:, :, :])
            wpr_sb = const.tile([P, 8, C], BF16)
            nc.sync.dma_start(wpr_sb, wpr[:, :, :])

            # ---- emission helpers ----
            def emit_qk_ft(x_sb, roped, ft):
                ps = psC.tile([P, 512], F32, tag="chain")
                fg, half = ft // 2, ft % 2
                for kt in range(8):
                    nc.tensor.matmul(
                        ps[:, 0:W],
                        wqk_sb[:, fg, kt, half * 128 : (half + 1) * 128],
                        x_sb[:, kt, :],
                        start=(kt == 0),
                        stop=(kt == 7),
                    )
                tmp = wk.tile([P, W], BF16, tag="tmp")
                nc.scalar.activation(
                    tmp, ps[:, 0:W], Ident, bias=bqk_sb[:, ft : ft + 1]
                )
                psr = psR.tile([P, 512], F32, tag="rope")
                nc.tensor.matmul(psr[:, 0:W], pmat_sb, tmp, start=True, stop=True)
                acc = wk.tile([P, W], BF16, tag="acc")
                nc.gpsimd.tensor_mul(acc, tmp, cos_sb)
                rot2 = wk.tile([P, W], BF16, tag="rot2")
                nc.vector.tensor_mul(rot2, psr[:, 0:W], sin_sb)
                nc.gpsimd.tensor_add(roped[:, ft, :], acc, rot2)

            def make_vt(it2, tt):
                pcount = 128 if tt == 0 else 69
                vt = vp.tile([P, 16, 128], BF16, tag=f"v{it2}{tt}")
                return vt

            def emit_v_nk(x_sb, vt, it2, tt, nk, kts=range(8), ps=None):
                pcount = 128 if tt == 0 else 69
                kts = list(kts)
                if ps is None:
                    ps = psC.tile([P, 512], F32, tag="chain")
                for kt in kts:
                    nc.tensor.matmul(
                        ps[:pcount, :],
                        x_sb[:, kt, it2 * S + tt * P : it2 * S + tt * P + pcount],
                        wv_sb[:, kt, nk * 512 : (nk + 1) * 512],
                        start=(kt == 0),
                        stop=(kt == 7),
                    )
                if kts[-1] != 7:
                    return ps
                # ones in cols 0:64, features in 64:128 so the AV output has
                # the softmax sums at partition base 0 (reciprocal_approx_fast
                # mis-reads partition-offset inputs on HW)
                nc.vector.tensor_copy(
                    vt[:pcount, nk * 8 : (nk + 1) * 8, 64:128],
                    ps[:pcount, :].rearrange("p (h d) -> p h d", d=64),
                )
                if nk == 1:
                    nc.gpsimd.memset(vt[:pcount, :, 0:64], 1.0)

            def emit_proj_fused(concat, y, ft, kts=range(8), ps=None):
                """proj for BOTH items of a pair at once: N=W columns, so
                each weight tile is loaded once for 394 output columns
                (halves LDWEIGHTS traffic vs per-item chains)."""
                kts = list(kts)
                if ps is None:
                    ps = psC.tile([P, 512], F32, tag="chain")
                for kt in kts:
                    nc.tensor.matmul(
                        ps[:, 0:W],
                        wpr_sb[:, kt, ft * 128 : (ft + 1) * 128],
                        concat[:, kt, :],
                        start=(kt == 0),
                        stop=(kt == 7),
                    )
                if kts[-1] == 7:
                    nc.scalar.activation(
                        y[:, ft, :], ps[:, 0:W], Ident, bias=beff_sb[:, ft : ft + 1]
                    )
                return ps

            def emit_proj(concat, y, it2, ft, kts=range(8), ps=None):
                ts = it2 * S
                kts = list(kts)
                if ps is None:
                    ps = psC.tile([P, 512], F32, tag="chain")
                for kt in kts:
                    nc.tensor.matmul(
                        ps[:, 0:S],
                        wpr_sb[:, kt, ft * 128 : (ft + 1) * 128],
                        concat[:, kt, ts : ts + S],
                        start=(kt == 0),
                        stop=(kt == 7),
                    )
                if kts[-1] == 7:
                    nc.scalar.activation(
                        y[:, ft, it2, :], ps[:, 0:S], Ident,
                        bias=beff_sb[:, ft : ft + 1],
                    )
                return ps

            def emit_wave_sc(roped, it2, hp):
                """Scores for head pair hp. Per head one PSUM bank holding
                jt0 at cols 0:S and jt1 at cols 256:256+S. Both writes are
                full overwrites of disjoint ranges (start=True stop=True),
                never accumulation, so sharing the bank is safe on HW (start
                clears has_written bits, not data)."""
                ts = it2 * S
                sc = psS.tile([P, 1024], F32, tag="sc")
                kqs = []
                for bk, h in ((0, 2 * hp), (1, 2 * hp + 1)):
                    hb = 64 * (h % 2)
                    kT = roped[hb : hb + 64, 8 + h // 2, ts : ts + S]
                    qT = roped[hb : hb + 64, h // 2, ts : ts + S]
                    kqs.append((bk, kT, qT))
                # jt0 of both heads first so e0 is ready one matmul earlier
                for bk, kT, qT in kqs:
                    nc.tensor.matmul(
                        sc[:, bk * 512 : bk * 512 + S], kT[:, 0:P], qT,
                        start=True, stop=True, skip_group_check=True,
                    )
                for bk, kT, qT in kqs:
                    nc.tensor.matmul(
                        sc[0:69, bk * 512 + 256 : bk * 512 + 256 + S],
                        kT[:, P:S], qT,
                        start=True, stop=True, skip_group_check=True,
                    )
                sc4 = sc.rearrange("p (b c) -> p b c", b=2)
                # ONE exp spanning jt0+jt1 (cols 0:453) of both banks: AV's
                # jt1 matmuls then wait one EXP (~830ns) instead of two
                # serial ones (~1170ns). The pad cols 197:256 and rows 69:128
                # of the jt1 range hold stale psum -> exp garbage in unused
                # e regions, harmless on HW.
                e = ep.tile([P, 2, 256 + S], BF16, tag="e")
                nc.scalar.activation(e, sc4[:, :, 0 : 256 + S], Exp, scale=0.125)
                return e[:, :, 0:S], e[:, :, 256 : 256 + S]

            def emit_wave_av(v65, concat, e0, e1, it2, hp):
                """AV + normalization for a head pair; per head one standard
                2-matmul accumulation group in its own bank."""
                ts = it2 * S
                hA, hB = 2 * hp, 2 * hp + 1
                av = psV.tile([P, 1024], F32, tag="av")
                nc.tensor.matmul(
                    av[:, 0:S], v65[it2][0][:, hA, :], e0[:, 0],
                    start=True, stop=False,
                )
                nc.tensor.matmul(
                    av[:, 512 : 512 + S], v65[it2][0][:, hB, :], e0[:, 1],
                    start=True, stop=False,
                )
                nc.tensor.matmul(
                    av[:, 0:S], v65[it2][1][0:69, hA, :], e1[0:69, 0],
                    start=False, stop=True,
                )
                nc.tensor.matmul(
                    av[:, 512 : 512 + S], v65[it2][1][0:69, hB, :], e1[0:69, 1],
                    start=False, stop=True,
                )
                av4 = av.rearrange("p (b c) -> p b c", b=2)
                rb = wk.tile([64, 2, S], F32, tag="rb", bufs=1)
                nc.vector.reciprocal_approx_fast(rb, av4[0:64, :, 0:S])
                nc.vector.tensor_mul(
                    concat[0:64, hp, ts : ts + S], av4[64:128, 0, 0:S], rb[:, 0]
                )
                nc.vector.tensor_mul(
                    concat[64:128, hp, ts : ts + S], av4[64:128, 1, 0:S], rb[:, 1]
                )

            # ---- prologue: pair-0 QKV + V(it0), bare (DMA-paced anyway) ----
            roped_cur = rp.tile([P, 16, W], BF16)
            for ft in range(16):
                emit_qk_ft(x_first, roped_cur, ft)
            v65_cur = [[None, None], [None, None]]
            for tt in range(2):
                v65_cur[0][tt] = make_vt(0, tt)
                for nk in range(2):
                    emit_v_nk(x_first, v65_cur[0][tt], 0, tt, nk)

            # ---- software-pipelined pair phases with a filler-unit queue ----
            queue = deque()
            x_cur = x_first

            def enq_split(fn):
                """Enqueue a chain as two 4-matmul halves sharing one psum
                tile (finer filler granularity for slot-starved phases)."""
                sh = {}

                def a():
                    sh["ps"] = fn(kts=range(0, 4))

                def b():
                    fn(kts=range(4, 8), ps=sh["ps"])

                queue.append(a)
                queue.append(b)

            def enq_v_units(x_sb, v65dst, split):
                for it2 in range(2):
                    for tt in range(2):
                        v65dst[it2][tt] = make_vt(it2, tt)
                        for nk in range(2):
                            fn = (
                                lambda x=x_sb, vt=v65dst[it2][tt], i=it2,
                                t=tt, n=nk, **kw: emit_v_nk(x, vt, i, t, n, **kw)
                            )
                            if split:
                                enq_split(fn)
                            else:
                                queue.append(fn)

            concat_prev = None
            yF_prev = None

            for pr in range(NPAIR):
                nxt = pr + 1 if pr + 1 < NPAIR else None
                concat = cc.tile([P, 8, W], BF16)

                if pr == 0:
                    # v65 for pair-0 it1, consumed by the first it0 pops
                    for tt in range(2):
                        v65_cur[1][tt] = make_vt(1, tt)
                        for nk in range(2):
                            queue.append(
                                (lambda x=x_cur, vt=v65_cur[1][tt], t=tt, n=nk:
                                 emit_v_nk(x, vt, 1, t, n))
                            )

                if nxt is not None:
                    x_nxt = xp.tile([P, 8, W], BF16, tag="x")
                    nc.sync.dma_start(x_nxt, xT[:, nxt])
                    roped_nxt = rp.tile([P, 16, W], BF16)
                    for ft in range(16):
                        queue.append(
                            (lambda x=x_nxt, r=roped_nxt, f=ft:
                             emit_qk_ft(x, r, f))
                        )
                else:
                    # final phase it0 filler: fused proj of the previous pair
                    # as WHOLE chains, one per scores->AV slot (a 1.4us chain
                    # covers the exp latency; half-chains spread over both
                    # pop points undershoot it)
                    yF = yp.tile([P, 8, W], F32, tag="ybig", bufs=2)
                    for ft in range(8):
                        queue.append(
                            (lambda c=concat_prev, y=yF, f=ft:
                             emit_proj_fused(c, y, f))
                        )
                    yF_prev = yF

                # it0 waves; in the last phase only the scores->AV pop is used
                for hp in range(8):
                    e0, e1 = emit_wave_sc(roped_cur, 0, hp)
                    if queue:
                        queue.popleft()()
                    emit_wave_av(v65_cur, concat, e0, e1, 0, hp)
                    if queue and nxt is not None:
                        queue.popleft()()

                if nxt is not None:
                    v65_nxt = [[None, None], [None, None]]
                    enq_v_units(x_nxt, v65_nxt, split=(pr == 0))
                    if pr >= 1:
                        # fused proj of pair pr-1 (whole chains)
                        yF = yp.tile([P, 8, W], F32, tag="ybig", bufs=2)
                        for ft in range(8):
                            queue.append(
                                (lambda c=concat_prev, y=yF, f=ft:
                                 emit_proj_fused(c, y, f))
                            )
                        yF_prev = yF
                else:
                    v65_nxt = None
                    # deferred pair-2 output is fully emitted by now
                    nc.sync.dma_start(
                        out[:, NPAIR - 2],
                        yF_prev.rearrange("p f (it s) -> p f it s", it=2),
                    )
                    # final phase it1 filler: this pair's it0 proj as whole
                    # chains, one per scores->AV slot
                    y3big = yp.tile([P, 8, W], F32, tag="ybig", bufs=2)
                    y3 = y3big.rearrange("p f (it s) -> p f it s", it=2)
                    for ft in range(8):
                        queue.append(
                            (lambda c=concat, y=y3, f=ft:
                             emit_proj(c, y, 0, f))
                        )

                # it1 waves
                for hp in range(8):
                    e0, e1 = emit_wave_sc(roped_cur, 1, hp)
                    if queue:
                        queue.popleft()()
                    emit_wave_av(v65_cur, concat, e0, e1, 1, hp)
                    if queue and nxt is not None:
                        queue.popleft()()

                # drain leftovers (pure chain work, gap-free)
                while queue:
                    queue.popleft()()

                if debug_taps and pr == 0:
                    nc.sync.dma_start(
                        dbg_roped.rearrange("p (f w) -> p f w", f=16), roped_cur
                    )
                    nc.sync.dma_start(
                        dbg_v.rearrange("p (h d) -> p h d", h=16), v65_cur[0][0]
                    )
                    nc.sync.dma_start(
                        dbg_concat.rearrange("p (j w) -> p j w", j=8), concat
                    )
                    nc.sync.dma_start(dbg_e[:, :], e0[:, 0])

                if nxt is not None:
                    if pr >= 1:
                        # pair pr-1's fused output is complete after the drain
                        nc.sync.dma_start(
                            out[:, pr - 1],
                            yF_prev.rearrange("p f (it s) -> p f it s", it=2),
                        )
                    concat_prev = concat
                    x_cur, roped_cur, v65_cur = x_nxt, roped_nxt, v65_nxt
                else:
                    # tail: this pair's it0 output, then bare it1 proj chains
                    # with the output streamed out in 2-ft chunks so the last
                    # DMA only moves ~200KB after the final IDENT
                    nc.sync.dma_start(out[:, pr, :, 0, :], y3[:, :, 0])
                    for ft in range(8):
                        emit_proj(concat, y3, 1, ft)
                        if ft % 2 == 1:
                            nc.sync.dma_start(
                                out[:, pr, ft - 1 : ft + 1, 1, :],
                                y3[:, ft - 1 : ft + 1, 1],
                            )

    nc.compile()
    return nc


def host_pack_inputs(x, qkv_w, qkv_b, proj_w, proj_b, n_items=NI):
    """Build per-core in_maps (host-side layout only, no math on x)."""
    perm, cosT, sinST, pmatT = _host_tables()
    wqk_dev, wv_dev, wpr_dev, bqk_dev, beff_dev = _pack_weights(
        qkv_w, qkv_b, proj_w, proj_b, perm
    )
    shared = {
        "wqk": np.ascontiguousarray(wqk_dev.reshape(P, -1)),
        "wv": np.ascontiguousarray(wv_dev.reshape(P, -1)),
        "wpr": np.ascontiguousarray(wpr_dev.reshape(P, -1)),
        "pmat": np.ascontiguousarray(pmatT),
        "bqk": bqk_dev,
        "beff": beff_dev,
        "cosT": np.ascontiguousarray(cosT),
        "sinST": np.ascontiguousarray(sinST),
    }
    n_cores = x.shape[0] // n_items
    in_maps = []
    for c in range(n_cores):
        xs = x[c * n_items : (c + 1) * n_items]  # [ni, 197, 1024]
        # device layout [p, pair, kt, w]: feature kt*128+p, token pair*394+w
        xt = xs.reshape(NPAIR, W, C).astype(BF16_NP)
        xt = xt.transpose(2, 0, 1).reshape(8, 128, NPAIR, W).transpose(1, 2, 0, 3)
        in_maps.append(
            {"xT": np.ascontiguousarray(xt.reshape(P, -1)), **shared}
        )
    return in_maps


def unpack_output(results, n_items=NI):
    """results: per-core {'out': [128, NPAIR, 2, 8, S]} -> full (B, N, C)."""
    outs = []
    for r in results:
        yT = r["out"].reshape(P, NPAIR, 8, 2, S)
        # feature o*128+p, token pr*394+it*197+s; device layout [p,pr,o,it,s]
        y = yT.transpose(1, 3, 4, 2, 0).reshape(n_items * S, C)
        outs.append(y.reshape(n_items, S, C))
    return np.concatenate(outs, axis=0)


_CACHED = {}


def kernel(x, qkv_w, qkv_b, proj_w, proj_b):
    from concourse.bass_utils import run_bass_kernel_spmd

    x = np.asarray(x, dtype=np.float32)
    qkv_w = np.asarray(qkv_w, dtype=np.float32)
    qkv_b = np.asarray(qkv_b, dtype=np.float32)
    proj_w = np.asarray(proj_w, dtype=np.float32)
    proj_b = np.asarray(proj_b, dtype=np.float32)

    if "nc" not in _CACHED:
        _CACHED["nc"] = build_nc(NI)
    nc = _CACHED["nc"]
    in_maps = host_pack_inputs(x, qkv_w, qkv_b, proj_w, proj_b, NI)
    res = run_bass_kernel_spmd(nc, in_maps, core_ids=list(range(N_CORES)))
    return unpack_output(res.results, NI).astype(np.float32)


if __name__ == "__main__":
    pass
